# revision 7
# baseline (speedup 1.0000x reference)
"""Multi-head attention kernel for 8 Trainium2 NeuronCores.

Problem: B=4, T=2048, DIM=1024, 16 heads, head_dim=64, additive causal mask.
  q,k,v = x@W{q,k,v}.T ; attn = softmax(q k^T/8 + mask) ; out = (attn v)@Wo.T

Sharding (no collectives): core i handles batch i//2 and head-group i%2
(8 heads).  Each core projects q/k/v for its 8 heads only (512 features,
no duplicated projection work), runs full causal attention for those heads,
and computes a partial output projection (contraction over its 512
features).  The host sums the two partial outputs per batch while
unsharding.

On-chip math:
 - Projections run as fp8(e4m3) DoubleRow matmuls (K=256 per instr, 0.5
   cyc/row) with a hi+lo 3-term split (x_hi*w_hi + x_lo*w_hi + x_hi*w_lo)
   for near-bf16 accuracy at 2x bf16 speed.  Weights are pre-scaled by 32
   so hi values sit in e4m3's sweet spot; the scale is folded into the
   exp() scale and into Wo.
 - Scores S[k,q] accumulate in PSUM fp32; exp via scalar engine with
   scale=1/(8*32*32); causal handled by skipping fully-masked 128-blocks
   and one tril-mask multiply per diagonal block.
 - AV uses P as the stationary operand and [V|1] as the moving operand
   (65 cols), giving full PE utilization and the softmax denominator for
   free in column 64.  Attention output lands as [q,hd] per (head,
   q-block), is normalized on DVE (denominator reciprocal broadcast), and
   PE-transposed back to [hd, q] for the output projection.
"""

import sys
import numpy as np

sys.path.insert(0, "/opt/trn_rl_repo")

import ml_dtypes  # noqa: E402
from contextlib import ExitStack  # noqa: E402
from concourse import bass, bacc, tile  # noqa: E402
from concourse.bass_utils import run_bass_kernel_spmd  # noqa: E402
from concourse.masks import make_identity  # noqa: E402

mybir = bass.mybir

B, T, DIM, H, HD = 4, 2048, 1024, 16, 64
HL = 8             # heads per core (head-group)
FG = 512           # features per core (HL * HD)
NJ = 4             # DoubleRow contraction chunks (256 each)
NTC = T // 128     # 16 key chunks
BF16 = mybir.dt.bfloat16
F8 = mybir.dt.float8e4
F32 = mybir.dt.float32
WS = 32.0          # weight pre-scale for e4m3
f8 = ml_dtypes.float8_e4m3fn
bf = ml_dtypes.bfloat16


# --------------------------------------------------------------------------
# fast causal path
# --------------------------------------------------------------------------

def _build_fast(reps=1):
    nc = bacc.Bacc("TRN2", target_bir_lowering=False, debug=False, num_devices=8)
    DR = mybir.MatmulPerfMode.DoubleRow
    Exp = mybir.ActivationFunctionType.Exp
    mult = mybir.AluOpType.mult

    xh_t = nc.dram_tensor("xh", [512, 4096], F8, kind="ExternalInput").ap()
    xl_t = nc.dram_tensor("xl", [512, 4096], F8, kind="ExternalInput").ap()
    w_t = {}
    for w in ("wq", "wk", "wv"):
        for p in ("h", "l"):
            w_t[w + p] = nc.dram_tensor(w + p, [512, 1024], F8, kind="ExternalInput").ap()
    wo_t = nc.dram_tensor("wo", [FG, DIM], BF16, kind="ExternalInput").ap()
    em_t = nc.dram_tensor("em", [128, 128], BF16, kind="ExternalInput").ap()
    y_t = nc.dram_tensor("y", [T, DIM], BF16, kind="ExternalOutput").ap()

    with tile.TileContext(nc) as tc:
      for _rep in range(reps):
        ctx = ExitStack()
        ctx.__enter__()

        # ---- pools -------------------------------------------------------
        x_p = ctx.enter_context(tc.tile_pool(name="xp", bufs=1))
        w_p = ctx.enter_context(tc.tile_pool(name="wp", bufs=1))
        wo_p = ctx.enter_context(tc.tile_pool(name="wop", bufs=1))
        kt_p = ctx.enter_context(tc.tile_pool(name="ktp", bufs=1))
        qt_p = ctx.enter_context(tc.tile_pool(name="qtp", bufs=1))
        va_p = ctx.enter_context(tc.tile_pool(name="vap", bufs=1))
        misc_p = ctx.enter_context(tc.tile_pool(name="miscp", bufs=1))
        otn_p = ctx.enter_context(tc.tile_pool(name="otnp", bufs=1))
        p_p = ctx.enter_context(tc.tile_pool(name="pp", bufs=4))
        nt_p = ctx.enter_context(tc.tile_pool(name="ntp", bufs=2))
        rec_p = ctx.enter_context(tc.tile_pool(name="recp", bufs=2))
        fin_p = ctx.enter_context(tc.tile_pool(name="finp", bufs=3))
        # PSUM: s 2x[128,1024] = 4 banks, o 2x[128,260] = 2 (shared with the
        # [128,128] transpose outputs), w 2x[128,512] = 2  -> 8 banks.
        psS = ctx.enter_context(tc.tile_pool(name="psS", bufs=2, space="PSUM"))
        psO = ctx.enter_context(tc.tile_pool(name="psO", bufs=2, space="PSUM"))
        psW = ctx.enter_context(tc.tile_pool(name="psW", bufs=2, space="PSUM"))

        # ---- persistent SBUF tiles --------------------------------------
        xh_sb = [x_p.tile([128, 2, T], F8, tag=f"xh{j}", name=f"xh{j}") for j in range(NJ)]
        xl_sb = [x_p.tile([128, 2, T], F8, tag=f"xl{j}", name=f"xl{j}") for j in range(NJ)]
        w_sb = {}
        for w in ("wq", "wk", "wv"):
            for p in ("h", "l"):
                w_sb[w + p] = [
                    w_p.tile([128, 2, FG], F8, tag=f"{w}{p}{j}", name=f"{w}{p}{j}")
                    for j in range(NJ)
                ]
        wo_sb = [wo_p.tile([128, DIM], BF16, tag=f"wo{i}", name=f"wo{i}") for i in range(4)]
        kt_sb = [kt_p.tile([128, T], BF16, tag=f"kt{i}", name=f"kt{i}") for i in range(4)]
        qt_sb = [qt_p.tile([128, T], BF16, tag=f"qt{i}", name=f"qt{i}") for i in range(4)]
        va_sb = [va_p.tile([128, HL, 65], BF16, tag=f"va{i}", name=f"va{i}") for i in range(NTC)]
        otn_sb = [otn_p.tile([128, T], BF16, tag=f"otn{i}", name=f"otn{i}") for i in range(4)]
        em_sb = misc_p.tile([128, 128], BF16, tag="em", name="em_sb")
        ident = misc_p.tile([128, 128], BF16, tag="id", name="ident")
        make_identity(nc, ident[:])

        # ---- input DMAs (in consumption order; Pool-dispatched: 25ns/DMA
        # vs 565ns on SP) -------------------------------------------------
        nc.gpsimd.dma_start(em_sb[:], em_t[:])

        def dma_w(w, p, j):
            nc.gpsimd.dma_start(
                w_sb[w + p][j][:].rearrange("p a b -> p (a b)"),
                w_t[w + p][j * 128:(j + 1) * 128, :],
            )

        def dma_x(p, j, tg):
            t, sb = (xh_t, xh_sb) if p == "h" else (xl_t, xl_sb)
            src = t[j * 128:(j + 1) * 128, :].rearrange(
                "p (i t) -> p i t", i=2
            )[:, :, tg * 512:(tg + 1) * 512]
            nc.gpsimd.dma_start(sb[j][:, :, tg * 512:(tg + 1) * 512], src)

        # hi pieces first, interleaved with x so the first K-tile matmuls
        # can start after ~2 DMAs
        for j in range(NJ):
            dma_w("wk", "h", j)
            dma_x("h", j, 0)
        for j in range(NJ):
            dma_w("wq", "h", j)
        for j in range(NJ):
            dma_w("wv", "h", j)
        for j in range(NJ):
            dma_x("l", j, 0)
        for w in ("wk", "wq", "wv"):
            for j in range(NJ):
                dma_w(w, "l", j)
        for j in range(NJ):
            dma_x("h", j, 1)
            dma_x("l", j, 1)
        for i in range(4):
            nc.gpsimd.dma_start(wo_sb[i][:], wo_t[i * 128:(i + 1) * 128, :])
        for tg in (2, 3):
            for j in range(NJ):
                dma_x("h", j, tg)
                dma_x("l", j, tg)

        for c in range(NTC):
            nc.vector.memset(va_sb[c][:, :, 64:65], 1.0)

        # ---- projection tile emitters -----------------------------------
        def kq_tile(w, dst, fc, tg):
            """K or Q projection tile: out [128 feat, 512 tok]."""
            ps = psW.tile([128, 512], F32, tag="w", name=f"{w}_ps")
            i = 0
            for (xs, ws) in ((xh_sb, w_sb[w + "h"]), (xl_sb, w_sb[w + "h"]),
                             (xh_sb, w_sb[w + "l"])):
                for j in range(NJ):
                    nc.tensor.matmul(
                        ps[:],
                        ws[j][:, :, fc * 128:(fc + 1) * 128],
                        xs[j][:, :, tg * 512:(tg + 1) * 512],
                        start=(i == 0), stop=(i == 3 * NJ - 1), perf_mode=DR,
                    )
                    i += 1
            nc.vector.tensor_copy(dst[fc][:, tg * 512:(tg + 1) * 512], ps[:])

        def v_tile(c):
            """V projection for key chunk c: out [128 tok, 512 feat]."""
            ps = psW.tile([128, 512], F32, tag="w", name="v_ps")
            i = 0
            for (xs, ws) in ((xh_sb, w_sb["wvh"]), (xl_sb, w_sb["wvh"]),
                             (xh_sb, w_sb["wvl"])):
                for j in range(NJ):
                    nc.tensor.matmul(
                        ps[:],
                        xs[j][:, :, c * 128:(c + 1) * 128],
                        ws[j][:],
                        start=(i == 0), stop=(i == 3 * NJ - 1), perf_mode=DR,
                    )
                    i += 1
            nc.vector.tensor_copy(
                va_sb[c][:, :, 0:64],
                ps[:].rearrange("p (h d) -> p h d", h=HL),
            )

        def o_group(tb, half):
            """Output projection for one [128 tok, 512 out] block."""
            ps = psW.tile([128, 512], F32, tag="w", name="f_ps")
            for fc in range(4):
                nc.tensor.matmul(
                    ps[:],
                    otn_sb[fc][:, tb * 128:(tb + 1) * 128],
                    wo_sb[fc][:, half * 512:(half + 1) * 512],
                    start=(fc == 0), stop=(fc == 3),
                )
            fin = fin_p.tile([128, 512], BF16, tag="fin", name="fin")
            nc.vector.tensor_copy(fin[:], ps[:])
            nc.sync.dma_start(
                y_t[tb * 128:(tb + 1) * 128, half * 512:(half + 1) * 512], fin[:]
            )

        # ---- filler schedule (deadline-ordered) -------------------------
        # slots[qg][h] = list of thunks to emit after head h of group qg.
        slots = [[[] for _ in range(HL)] for _ in range(4)]

        def KQ(fc, tg):
            return [lambda: kq_tile("wk", kt_sb, fc, tg),
                    lambda: kq_tile("wq", qt_sb, fc, tg)]

        def V(c):
            return [lambda c=c: v_tile(c)]

        def O(tb):
            return [lambda: o_group(tb, 0), lambda: o_group(tb, 1)]

        # kt/qt for (fc, tg0) beyond fc0, woven inside qg0 ahead of use
        slots[0][0] += KQ(1, 0)
        slots[0][1] += KQ(2, 0)
        slots[0][2] += KQ(3, 0)
        # next-group projections + V chunks, all due before next qg starts
        for qg in range(3):
            ntg = qg + 1
            slots[qg][3] += KQ(0, ntg)
            slots[qg][4] += KQ(1, ntg)
            slots[qg][5] += KQ(2, ntg) + V(4 * ntg)
            slots[qg][6] += KQ(3, ntg) + V(4 * ntg + 1) + V(4 * ntg + 2)
            slots[qg][7] += V(4 * ntg + 3)
        # output projection of qg woven into qg+1
        for qg in range(3):
            for tb in range(4):
                slots[qg + 1][tb + 1] += O(4 * qg + tb)

        # ---- pre-phase: minimum tiles for (qg0, h0) ---------------------
        kq_tile("wk", kt_sb, 0, 0)
        kq_tile("wq", qt_sb, 0, 0)
        for c in range(4):
            v_tile(c)

        # ---- attention ---------------------------------------------------
        def attention(hl, qg):
            fc, hr = hl // 2, (hl % 2) * 64
            nch = 4 * qg + 4
            qsl = slice(qg * 512, (qg + 1) * 512)
            o_ps = psO.tile([128, 260], F32, tag="o", name="o_ps")
            o4 = o_ps[:].rearrange("p (q x) -> p q x", q=4)
            prev = None  # (p_tile, base_c)

            def do_av(p_tile, base_c):
                # all four 65-col regions share one PSUM bank: exactly one
                # start (zeroes the bank) and one stop for the whole tile
                for cc in (base_c, base_c + 1):
                    hh = cc - base_c
                    M = max(0, cc - 4 * qg)
                    for qb in range(M, 4):
                        nc.tensor.matmul(
                            o4[:, qb, :],
                            p_tile[:, hh * 512 + qb * 128: hh * 512 + (qb + 1) * 128],
                            va_sb[cc][:, hl, :],
                            start=(cc == 0 and qb == 0),
                            stop=(cc == nch - 1 and qb == 3),
                        )

            for base_c in range(0, nch, 2):
                s_ps = psS.tile([128, 1024], F32, tag="s", name="s_ps")
                p_tile = p_p.tile([128, 1024], BF16, tag="p", name="p_tile")
                Ms = []
                for hh in (0, 1):
                    cc = base_c + hh
                    M = max(0, cc - 4 * qg)
                    Ms.append(M)
                    nc.tensor.matmul(
                        s_ps[:, hh * 512 + M * 128: (hh + 1) * 512],
                        kt_sb[fc][hr:hr + 64, cc * 128:(cc + 1) * 128],
                        qt_sb[fc][hr:hr + 64, qg * 512 + M * 128:(qg + 1) * 512],
                        start=True, stop=True,
                    )
                if Ms[0] == Ms[1]:
                    M = Ms[0]
                    s_act = s_ps[:].rearrange("p (h x) -> p h x", h=2)[
                        :, :, M * 128: 512]
                    p_act = p_tile[:].rearrange("p (h x) -> p h x", h=2)[
                        :, :, M * 128: 512]
                    nc.scalar.activation(p_act, s_act, Exp, scale=1.0 / (8 * WS * WS))
                else:
                    for hh in (0, 1):
                        M = Ms[hh]
                        sl = slice(hh * 512 + M * 128, (hh + 1) * 512)
                        nc.scalar.activation(
                            p_tile[:, sl], s_ps[:, sl], Exp, scale=1.0 / (8 * WS * WS))
                # diagonal-block mask multiply
                for hh in (0, 1):
                    cc = base_c + hh
                    dqb = cc - 4 * qg
                    if 0 <= dqb < 4:
                        sl = slice(hh * 512 + dqb * 128, hh * 512 + (dqb + 1) * 128)
                        nc.gpsimd.tensor_tensor(
                            p_tile[:, sl], p_tile[:, sl], em_sb[:], mult)
                if prev is not None:
                    do_av(*prev)
                prev = (p_tile, base_c)
            do_av(*prev)

            # normalize: rec of denominators (col 64 of each 65-group)
            rec4 = rec_p.tile([128, 4], F32, tag="rec", name="rec4")
            nc.vector.reciprocal(rec4[:].unsqueeze(-1), o4[:, :, 64:65])
            if hl % 2 == 0:
                nt4 = nt_p.tile([128, 4, 128], BF16, tag="nt", name="nt4")
                attention.nt4 = nt4
            else:
                nt4 = attention.nt4
            nc.vector.tensor_tensor(
                nt4[:, :, hr:hr + 64],
                o4[:, :, 0:64],
                rec4[:].unsqueeze(-1).broadcast_to((128, 4, 64)),
                mult,
            )
            if hl % 2 == 1:
                for qb in range(4):
                    pt = psO.tile([128, 128], BF16, tag="o", name="pst")
                    nc.tensor.transpose(pt[:], nt4[:, qb, :], ident[:])
                    nc.vector.tensor_copy(
                        otn_sb[fc][:, qg * 512 + qb * 128: qg * 512 + (qb + 1) * 128],
                        pt[:],
                    )

        for qg in range(4):
            for hl in range(HL):
                attention(hl, qg)
                for task in slots[qg][hl]:
                    task()
        for tb in range(4):
            o_group(12 + tb, 0)
            o_group(12 + tb, 1)

        ctx.__exit__(None, None, None)

    nc.compile()
    return nc


# --------------------------------------------------------------------------
# host-side prep for the fast path
# --------------------------------------------------------------------------

def _q8(a):
    hi = a.astype(f8)
    lo = (a - hi.astype(np.float32)).astype(f8)
    return hi, lo


def _dr_layout(a):
    """[1024, N] contraction-major -> [512, 2N] DoubleRow layout.

    Row r = j*128 + p holds contraction indices d = 256j + 128i + p in
    column halves i = 0, 1 (per 128-row block).
    """
    n = a.shape[1]
    return np.ascontiguousarray(
        a.reshape(NJ, 2, 128, n).transpose(0, 2, 1, 3).reshape(512, 2 * n)
    )


def _prep_fast_inputs(x, wq, wk, wv, wo):
    ins = []
    # p_tile is [key, query]: allowed iff key <= query -> upper triangular
    em = np.triu(np.ones((128, 128), dtype=np.float32)).astype(bf)
    xq = []
    for b in range(B):
        xh, xl = _q8(x[b].T)  # [1024, 2048]
        xq.append((_dr_layout(xh), _dr_layout(xl)))
    wqs = []
    for g in range(2):
        d = {}
        for name, w in (("wq", wq), ("wk", wk), ("wv", wv)):
            wt = np.ascontiguousarray(w[g * FG:(g + 1) * FG, :].T) * WS  # [1024, 512]
            hi, lo = _q8(wt)
            d[name + "h"] = _dr_layout(hi)
            d[name + "l"] = _dr_layout(lo)
        d["wo"] = np.ascontiguousarray(wo[:, g * FG:(g + 1) * FG].T / WS).astype(bf)
        wqs.append(d)
    for i in range(8):
        b, g = i // 2, i % 2
        im = {"xh": xq[b][0], "xl": xq[b][1], "em": em}
        im.update(wqs[g])
        ins.append(im)
    return ins


# --------------------------------------------------------------------------
# legacy general/nomask path (baseline kernel, kept as fallback)
# --------------------------------------------------------------------------

TQ = 1024
NDC = DIM // 128


def _causal_sched():
    sched = []
    for a in (0, 1):
        for c in range(8 * a + 8):
            sched.append((a, c, (c - 8 * a) // 2))
    return sched


CAUSAL_SCHED = _causal_sched()


def _build_legacy(variant, reps=1):
    """variant: 'general' | 'nomask' (original baseline kernel)."""
    nc = bacc.Bacc("TRN2", target_bir_lowering=False, debug=False, num_devices=8)

    xT = nc.dram_tensor("xT", [DIM, T], BF16, kind="ExternalInput").ap()
    xqT = nc.dram_tensor("xqT", [DIM, TQ], BF16, kind="ExternalInput").ap()
    wqT = nc.dram_tensor("wqT", [DIM, DIM], BF16, kind="ExternalInput").ap()
    wkT = nc.dram_tensor("wkT", [DIM, DIM], BF16, kind="ExternalInput").ap()
    wvT = nc.dram_tensor("wvT", [DIM, DIM], BF16, kind="ExternalInput").ap()
    woT = nc.dram_tensor("woT", [DIM, DIM], BF16, kind="ExternalInput").ap()
    if variant == "general":
        em = nc.dram_tensor("em", [T, TQ], BF16, kind="ExternalInput").ap()
    else:
        em = None
    out = nc.dram_tensor("out", [TQ, DIM], F32, kind="ExternalOutput").ap()

    with tile.TileContext(nc) as tc:
      for _rep in range(reps):
        ctx = ExitStack()
        ctx.__enter__()
        Exp = mybir.ActivationFunctionType.Exp
        mult = mybir.AluOpType.mult

        qt_p = ctx.enter_context(tc.tile_pool(name="qt", bufs=1))
        kt_p = ctx.enter_context(tc.tile_pool(name="kt", bufs=1))
        va_p = ctx.enter_context(tc.tile_pool(name="va", bufs=1))
        misc_p = ctx.enter_context(tc.tile_pool(name="misc", bufs=1))
        psS = ctx.enter_context(tc.tile_pool(name="psS", bufs=2, space="PSUM"))
        psO = ctx.enter_context(tc.tile_pool(name="psO", bufs=1, space="PSUM"))
        psB = ctx.enter_context(tc.tile_pool(name="psB", bufs=1, space="PSUM"))
        psF = ctx.enter_context(tc.tile_pool(name="psF", bufs=1, space="PSUM"))

        qt_sb = [qt_p.tile([128, TQ], BF16, tag=f"qt{i}", name=f"qt{i}") for i in range(NDC)]
        kt_sb = [kt_p.tile([128, T], BF16, tag=f"kt{i}", name=f"kt{i}") for i in range(NDC)]
        va_sb = [va_p.tile([128, H * 65], BF16, tag=f"va{i}", name=f"va{i}") for i in range(NTC)]
        ones_sb = misc_p.tile([128, 64], F32, tag="ones", name="ones")
        nc.vector.memset(ones_sb[64:65, :], 1.0)

        with tc.tile_pool(name="xin", bufs=1) as x_p, tc.tile_pool(
            name="win", bufs=1
        ) as w_p:
            xt_sb = [x_p.tile([128, T], BF16, tag=f"xt{i}", name=f"xt{i}") for i in range(NDC)]
            xq_sb = [x_p.tile([128, TQ], BF16, tag=f"xq{i}", name=f"xq{i}") for i in range(NDC)]
            wq_sb = [w_p.tile([128, DIM], BF16, tag=f"wq{i}", name=f"wq{i}") for i in range(NDC)]
            wk_sb = [w_p.tile([128, DIM], BF16, tag=f"wk{i}", name=f"wk{i}") for i in range(NDC)]
            wv_sb = [w_p.tile([128, DIM], BF16, tag=f"wv{i}", name=f"wv{i}") for i in range(NDC)]
            for i in range(NDC):
                s = slice(i * 128, (i + 1) * 128)
                nc.sync.dma_start(wv_sb[i][:], wvT[s, :])
                nc.sync.dma_start(xt_sb[i][:], xT[s, :])
            for i in range(NDC):
                s = slice(i * 128, (i + 1) * 128)
                nc.sync.dma_start(wq_sb[i][:], wqT[s, :])
                nc.sync.dma_start(xq_sb[i][:], xqT[s, :])
            for i in range(NDC):
                s = slice(i * 128, (i + 1) * 128)
                nc.sync.dma_start(wk_sb[i][:], wkT[s, :])

            for c in range(NTC):
                v4 = va_sb[c][:].rearrange("p (q t x) -> p q t x", q=8, t=2)
                nc.vector.memset(v4[:, :, 0:2, 64:65], 1.0)

            for c in range(NTC):
                csl = slice(c * 128, (c + 1) * 128)
                ps = psS.tile([128, 1024], F32, tag="s", name="v_ps")
                for n in range(2):
                    nsl = slice(n * 512, (n + 1) * 512)
                    for dc in range(NDC):
                        nc.tensor.matmul(
                            ps[:, nsl],
                            xt_sb[dc][:, csl],
                            wv_sb[dc][:, nsl],
                            start=(dc == 0),
                            stop=(dc == NDC - 1),
                        )
                v4 = va_sb[c][:].rearrange("p (q t x) -> p q t x", q=8, t=2)
                s4 = ps[:].rearrange("p (q t x) -> p q t x", q=8, t=2)
                nc.vector.tensor_copy(v4[:, :, 0:2, 0:64], s4[:, :, 0:2, :])

            for oc in range(NDC):
                osl = slice(oc * 128, (oc + 1) * 128)
                ps = psS.tile([128, 1024], F32, tag="s", name="q_ps")
                for n in range(2):
                    nsl = slice(n * 512, (n + 1) * 512)
                    for dc in range(NDC):
                        nc.tensor.matmul(
                            ps[:, nsl],
                            wq_sb[dc][:, osl],
                            xq_sb[dc][:, nsl],
                            start=(dc == 0),
                            stop=(dc == NDC - 1),
                        )
                nc.scalar.copy(qt_sb[oc][:], ps[:])
                for m in range(2):
                    ps = psS.tile([128, 1024], F32, tag="s", name="k_ps")
                    for n in range(2):
                        nsl = slice((2 * m + n) * 512, (2 * m + n + 1) * 512)
                        psl = slice(n * 512, (n + 1) * 512)
                        for dc in range(NDC):
                            nc.tensor.matmul(
                                ps[:, psl],
                                wk_sb[dc][:, osl],
                                xt_sb[dc][:, nsl],
                                start=(dc == 0),
                                stop=(dc == NDC - 1),
                            )
                    nc.scalar.copy(
                        kt_sb[oc][:, m * 1024: (m + 1) * 1024], ps[:]
                    )

        em_p = ctx.enter_context(tc.tile_pool(name="em", bufs=1))
        wo_p = ctx.enter_context(tc.tile_pool(name="wo", bufs=1))
        otn_p = ctx.enter_context(tc.tile_pool(name="otn", bufs=1))
        p_p = ctx.enter_context(tc.tile_pool(name="pp", bufs=4))
        osb_p = ctx.enter_context(tc.tile_pool(name="osb", bufs=2))
        nrm_p = ctx.enter_context(tc.tile_pool(name="nrm", bufs=2))
        fin_p = ctx.enter_context(tc.tile_pool(name="fin", bufs=2))

        if variant == "general":
            em_sb = [em_p.tile([128, TQ], BF16, tag=f"em{i}", name=f"emt{i}") for i in range(NTC)]
            for c in range(NTC):
                nc.sync.dma_start(em_sb[c][:], em[c * 128: (c + 1) * 128, :])
        wo_sb = [wo_p.tile([128, DIM], BF16, tag=f"wo{i}", name=f"wot{i}") for i in range(NDC)]
        for i in range(NDC):
            nc.sync.dma_start(wo_sb[i][:], woT[i * 128: (i + 1) * 128, :])
        otn_sb = [otn_p.tile([128, TQ], BF16, tag=f"otn{i}", name=f"otn{i}") for i in range(NDC)]
        otn1_sb = [otn_p.tile([64, TQ], BF16, tag=f"otn1{i}", name=f"otn1{i}") for i in range(NDC)]

        for a in (0, 1):
            nA = NTC
            asl = slice(a * 512, (a + 1) * 512)
            for hp in range(NDC):
                o_pair = psO.tile([128, 1024], F32, tag="opair", name="o_pair")
                for c in range(nA):
                    csl = slice(c * 128, (c + 1) * 128)
                    s_ps = psS.tile([128, 1024], F32, tag="s", name="s_ps")
                    nc.tensor.matmul(
                        s_ps[:, 0:512],
                        kt_sb[hp][0:64, csl],
                        qt_sb[hp][0:64, asl],
                        start=True,
                        stop=True,
                    )
                    nc.tensor.matmul(
                        s_ps[:, 512:1024],
                        kt_sb[hp][64:128, csl],
                        qt_sb[hp][64:128, asl],
                        start=True,
                        stop=True,
                        tile_position=(64, 0),
                    )
                    p_pair = p_p.tile([128, 1024], BF16, tag="p", name="p_pair")
                    nc.scalar.activation(p_pair[:], s_ps[:], Exp)
                    if variant == "general":
                        for half in (0, 1):
                            psl = slice(half * 512, half * 512 + 512)
                            nc.vector.tensor_mul(
                                p_pair[:, psl], p_pair[:, psl], em_sb[c][:, asl]
                            )
                    va4 = va_sb[c][:].rearrange("p (q t x) -> p q t x", q=8, t=2)
                    nc.tensor.matmul(
                        o_pair[0:65, 0:512],
                        va4[:, hp, 0, :],
                        p_pair[:, 0:512],
                        start=(c == 0),
                        stop=(c == nA - 1),
                    )
                    nc.tensor.matmul(
                        o_pair[0:65, 512:1024],
                        va4[:, hp, 1, :],
                        p_pair[:, 512:1024],
                        start=(c == 0),
                        stop=(c == nA - 1),
                    )
                o_sb = osb_p.tile([128, 1024], F32, tag="osb", name="o_sb")
                nc.vector.tensor_copy(o_sb[:], o_pair[:])
                rec = nrm_p.tile([128, 1024], mybir.dt.float32r, tag="rec", name="rec")
                with nc.allow_low_precision(reason="f32r recip for denom broadcast"):
                    nc.vector.reciprocal(rec[64:65, 0:1024], o_sb[64:65, 0:1024])
                b0 = psB.tile([64, 512], F32, tag="b", name="b0")
                b1 = psB.tile([64, 512], F32, tag="b", name="b1")
                nc.tensor.matmul(
                    b0[:],
                    ones_sb[64:65, :].bitcast(mybir.dt.float32r),
                    rec[64:65, 0:512],
                    start=True,
                    stop=True,
                    tile_position=(64, 0),
                )
                nc.tensor.matmul(
                    b1[:],
                    ones_sb[64:65, :].bitcast(mybir.dt.float32r),
                    rec[64:65, 512:1024],
                    start=True,
                    stop=True,
                    tile_position=(64, 0),
                )
                nc.vector.tensor_tensor(
                    otn_sb[hp][0:64, asl], o_sb[0:64, 0:512], b0[:], mult
                )
                nc.vector.tensor_tensor(
                    otn1_sb[hp][0:64, asl], o_sb[0:64, 512:1024], b1[:], mult
                )
                nc.sync.dma_start(otn_sb[hp][64:128, asl], otn1_sb[hp][0:64, asl])

            for tt in range(4 * a, 4 * a + 4):
                tsl = slice((tt % 4) * 128 + a * 512, (tt % 4) * 128 + a * 512 + 128)
                fin = fin_p.tile([128, 1024], F32, tag="fin", name="fin")
                for n in range(2):
                    nsl = slice(n * 512, (n + 1) * 512)
                    f_ps = psF.tile([128, 512], F32, tag="fwo", name="f_ps")
                    for hp in range(NDC):
                        nc.tensor.matmul(
                            f_ps[:],
                            otn_sb[hp][:, tsl],
                            wo_sb[hp][:, nsl],
                            start=(hp == 0),
                            stop=(hp == NDC - 1),
                        )
                    nc.vector.tensor_copy(fin[:, nsl], f_ps[:])
                nc.sync.dma_start(out[tt * 128: (tt + 1) * 128, :], fin[:])

        ctx.__exit__(None, None, None)

    nc.compile()
    return nc


_NC_CACHE = {}


def _get_nc(variant):
    if variant not in _NC_CACHE:
        if variant == "causal":
            _NC_CACHE[variant] = _build_fast()
        else:
            _NC_CACHE[variant] = _build_legacy(variant)
    return _NC_CACHE[variant]


def _kernel_legacy(x, emask, wq, wk, wv, wo, variant):
    scale = 1.0 / np.sqrt(HD)
    wqT = np.ascontiguousarray((wq * scale).T).astype(bf)
    wkT = np.ascontiguousarray(wk.T).astype(bf)
    wvT = np.ascontiguousarray(wv.T).astype(bf)
    woT = np.ascontiguousarray(wo.T).astype(bf)

    perms = {}
    for p in (0, 1):
        perms[p] = np.concatenate(
            [np.arange(128) + 128 * j for j in range(p, 16, 2)]
        )

    in_maps = []
    for i in range(8):
        b, p = i // 2, i % 2
        perm = perms[p]
        xb = x[b]
        im = {
            "xT": np.ascontiguousarray(xb.T).astype(bf),
            "xqT": np.ascontiguousarray(xb[perm].T).astype(bf),
            "wqT": wqT,
            "wkT": wkT,
            "wvT": wvT,
            "woT": woT,
        }
        if variant == "general":
            im["em"] = np.ascontiguousarray(emask.T[:, perm]).astype(bf)
        in_maps.append(im)

    nc = _get_nc(variant)
    res = run_bass_kernel_spmd(nc, in_maps, core_ids=list(range(8)))

    out_full = np.empty((B, T, DIM), dtype=np.float32)
    for i in range(8):
        b, p = i // 2, i % 2
        out_full[b, perms[p]] = res.results[i]["out"]
    return out_full


def kernel(x, mask, wq, wk, wv, wo):
    x = np.asarray(x, dtype=np.float32)
    mask = np.asarray(mask, dtype=np.float32)
    wq = np.asarray(wq, dtype=np.float32)
    wk = np.asarray(wk, dtype=np.float32)
    wv = np.asarray(wv, dtype=np.float32)
    wo = np.asarray(wo, dtype=np.float32)

    m2 = mask[0, 0]
    emask = np.exp(np.minimum(m2, 60.0)).astype(np.float32)
    tril = np.tril(np.ones((T, T), dtype=np.float32))
    if np.array_equal(emask, tril):
        variant = "causal"
    elif np.all(m2 == 0.0):
        variant = "nomask"
    else:
        variant = "general"

    if variant != "causal":
        return _kernel_legacy(x, emask, wq, wk, wv, wo, variant)

    in_maps = _prep_fast_inputs(x, wq, wk, wv, wo)
    nc = _get_nc("causal")
    res = run_bass_kernel_spmd(nc, in_maps, core_ids=list(range(8)))

    out_full = np.empty((B, T, DIM), dtype=np.float32)
    for b in range(B):
        y0 = res.results[2 * b]["y"].astype(np.float32)
        y1 = res.results[2 * b + 1]["y"].astype(np.float32)
        out_full[b] = y0 + y1
    return out_full


# revision 32
# speedup vs baseline: 1.2452x; 1.2452x over previous
"""Multi-head attention kernel for 8 Trainium2 NeuronCores.

Problem: B=4, T=2048, DIM=1024, 16 heads, head_dim=64, additive causal mask.
  q,k,v = x@W{q,k,v}.T ; attn = softmax(q k^T/8 + mask) ; out = (attn v)@Wo.T

Sharding (no collectives): core i handles batch i//2 and head-group i%2
(8 heads).  Each core projects q/k/v for its 8 heads only (512 features,
no duplicated projection work), runs full causal attention for those heads,
and computes a partial output projection (contraction over its 512
features).  The host sums the two partial outputs per batch while
unsharding.

On-chip math:
 - Projections run as fp8(e4m3) DoubleRow matmuls (K=256 per instr, 0.5
   cyc/row) with a hi+lo 3-term split (x_hi*w_hi + x_lo*w_hi + x_hi*w_lo)
   for near-bf16 accuracy at 2x bf16 speed.  Weights are pre-scaled by 32
   so hi values sit in e4m3's sweet spot; the scale is folded into the
   exp() scale and into Wo.
 - Scores S[k,q] accumulate in PSUM fp32; exp via scalar engine with
   scale=1/(8*32*32); causal handled by skipping fully-masked 128-blocks
   and one tril-mask multiply per diagonal block.
 - AV uses P as the stationary operand and [V|1] as the moving operand
   (65 cols), giving full PE utilization and the softmax denominator for
   free in column 64.  Attention output lands as [q,hd] per (head,
   q-block), is normalized on DVE (denominator reciprocal broadcast), and
   PE-transposed back to [hd, q] for the output projection.
"""

import sys
import numpy as np

sys.path.insert(0, "/opt/trn_rl_repo")

import ml_dtypes  # noqa: E402
from contextlib import ExitStack  # noqa: E402
from concourse import bass, bacc, tile  # noqa: E402
from concourse.bass_utils import run_bass_kernel_spmd  # noqa: E402
from concourse.masks import make_identity  # noqa: E402

mybir = bass.mybir

B, T, DIM, H, HD = 4, 2048, 1024, 16, 64
HL = 8             # heads per core (head-group)
FG = 512           # features per core (HL * HD)
NJ = 4             # DoubleRow contraction chunks (256 each)
NTC = T // 128     # 16 key chunks
BF16 = mybir.dt.bfloat16
F8 = mybir.dt.float8e4
F32 = mybir.dt.float32
WS = 32.0          # weight pre-scale for e4m3
f8 = ml_dtypes.float8_e4m3fn
bf = ml_dtypes.bfloat16


# --------------------------------------------------------------------------
# fast causal path
# --------------------------------------------------------------------------

def _build_fast(reps=1):
    nc = bacc.Bacc("TRN2", target_bir_lowering=False, debug=False, num_devices=8)
    DR = mybir.MatmulPerfMode.DoubleRow
    Exp = mybir.ActivationFunctionType.Exp
    mult = mybir.AluOpType.mult

    xh_t = nc.dram_tensor("xh", [128, NJ * 2 * T], F8, kind="ExternalInput").ap()
    xl_t = nc.dram_tensor("xl", [128, NJ * 2 * T], F8, kind="ExternalInput").ap()
    w_t = {}
    for w in ("wq", "wk", "wv"):
        for p in ("h", "l"):
            w_t[w + p] = nc.dram_tensor(w + p, [512, 1024], F8, kind="ExternalInput").ap()
    wo_t = nc.dram_tensor("wo", [FG, DIM], BF16, kind="ExternalInput").ap()
    em_t = nc.dram_tensor("em", [128, 128], BF16, kind="ExternalInput").ap()
    y_t = nc.dram_tensor("y", [T, DIM], BF16, kind="ExternalOutput").ap()

    with tile.TileContext(nc) as tc:
      for _rep in range(reps):
        ctx = ExitStack()
        ctx.__enter__()

        # ---- pools -------------------------------------------------------
        x_p = ctx.enter_context(tc.tile_pool(name="xp", bufs=1))
        w_p = ctx.enter_context(tc.tile_pool(name="wp", bufs=1))
        wo_p = ctx.enter_context(tc.tile_pool(name="wop", bufs=1))
        kt_p = ctx.enter_context(tc.tile_pool(name="ktp", bufs=1))
        qt_p = ctx.enter_context(tc.tile_pool(name="qtp", bufs=1))
        va_p = ctx.enter_context(tc.tile_pool(name="vap", bufs=1))
        misc_p = ctx.enter_context(tc.tile_pool(name="miscp", bufs=1))
        otn_p = ctx.enter_context(tc.tile_pool(name="otnp", bufs=1))
        p_p = ctx.enter_context(tc.tile_pool(name="pp", bufs=4))
        nt_p = ctx.enter_context(tc.tile_pool(name="ntp", bufs=2))
        rec_p = ctx.enter_context(tc.tile_pool(name="recp", bufs=2))
        fin_p = ctx.enter_context(tc.tile_pool(name="finp", bufs=3))
        # PSUM: s 2x[128,1024] = 4 banks, o 2x[128,260] = 2 (shared with the
        # [128,128] transpose outputs), w 2x[128,512] = 2  -> 8 banks.
        psS = ctx.enter_context(tc.tile_pool(name="psS", bufs=2, space="PSUM"))
        psO = ctx.enter_context(tc.tile_pool(name="psO", bufs=2, space="PSUM"))
        psW = ctx.enter_context(tc.tile_pool(name="psW", bufs=2, space="PSUM"))

        # ---- persistent SBUF tiles --------------------------------------
        xh4 = x_p.tile([128, NJ, 2, T], F8, tag="xh", name="xh4")
        xl4 = x_p.tile([128, NJ, 2, T], F8, tag="xl", name="xl4")
        w_sb = {}
        for w in ("wq", "wk", "wv"):
            for p in ("h", "l"):
                w_sb[w + p] = w_p.tile(
                    [128, NJ, 2, FG], F8, tag=f"{w}{p}", name=f"{w}{p}"
                )
        wo4 = wo_p.tile([128, 4, DIM], BF16, tag="wo", name="wo4")
        kt_sb = [kt_p.tile([128, T], BF16, tag=f"kt{i}", name=f"kt{i}") for i in range(4)]
        qt_sb = [qt_p.tile([128, T], BF16, tag=f"qt{i}", name=f"qt{i}") for i in range(4)]
        va_sb = [va_p.tile([128, HL, 65], BF16, tag=f"va{i}", name=f"va{i}") for i in range(NTC)]
        otn_sb = [otn_p.tile([128, T], BF16, tag=f"otn{i}", name=f"otn{i}") for i in range(4)]
        em_sb = misc_p.tile([128, 128], BF16, tag="em", name="em_sb")
        ident = misc_p.tile([128, 128], BF16, tag="id", name="ident")
        make_identity(nc, ident[:])

        # ---- input DMAs --------------------------------------------------
        # HWDGE serializes DMA issue (~650ns each), so FEW large DMAs in
        # strict first-needed order: K weights + x token-group 0 first.
        def dma_w(key):
            nc.sync.dma_start(
                w_sb[key][:].rearrange("p a b c -> p a (b c)"),
                w_t[key].rearrange("(j p) f -> p j f", j=NJ),
            )

        def dma_x(p, lo, hi):
            t, x4 = (xh_t, xh4) if p == "h" else (xl_t, xl4)
            src = t[:].rearrange("p (j i t) -> p j i t", j=NJ, i=2)
            nc.sync.dma_start(x4[:, :, :, lo:hi], src[:, :, :, lo:hi])

        dma_w("wkh")
        dma_x("h", 0, 512)
        dma_x("l", 0, 512)
        dma_w("wkl")
        dma_w("wqh")
        dma_w("wql")
        dma_w("wvh")
        dma_w("wvl")
        nc.sync.dma_start(em_sb[:], em_t[:])
        dma_x("h", 512, 2048)
        dma_x("l", 512, 2048)
        nc.sync.dma_start(
            wo4[:], wo_t[:].rearrange("(a p) o -> p a o", a=4),
        )

        for c in range(NTC):
            nc.vector.memset(va_sb[c][:, :, 64:65], 1.0)

        # ---- projection tile emitters (generators yielding every ~3
        # matmuls so filler can be woven at ~400ns granularity) -----------
        def kq_tile_gen(w, dst, fc, tg):
            """K or Q projection tile: out [128 feat, 512 tok]."""
            ps = psW.tile([128, 512], F32, tag="w", name=f"{w}_ps")
            i = 0
            for (xs, wk) in ((xh4, w + "h"), (xl4, w + "h"),
                             (xh4, w + "l")):
                for j in range(NJ):
                    nc.tensor.matmul(
                        ps[:],
                        w_sb[wk][:, j, :, fc * 128:(fc + 1) * 128],
                        xs[:, j, :, tg * 512:(tg + 1) * 512],
                        start=(i == 0), stop=(i == 3 * NJ - 1), perf_mode=DR,
                    )
                    i += 1
                    if i % 3 == 0 and i < 3 * NJ:
                        yield
            nc.vector.tensor_copy(dst[fc][:, tg * 512:(tg + 1) * 512], ps[:])

        def v_tile_gen(c):
            """V projection for key chunk c: out [128 tok, 512 feat]."""
            ps = psW.tile([128, 512], F32, tag="w", name="v_ps")
            i = 0
            for (xs, wk) in ((xh4, "wvh"), (xl4, "wvh"), (xh4, "wvl")):
                for j in range(NJ):
                    nc.tensor.matmul(
                        ps[:],
                        xs[:, j, :, c * 128:(c + 1) * 128],
                        w_sb[wk][:, j],
                        start=(i == 0), stop=(i == 3 * NJ - 1), perf_mode=DR,
                    )
                    i += 1
                    if i % 3 == 0 and i < 3 * NJ:
                        yield
            nc.vector.tensor_copy(
                va_sb[c][:, :, 0:64],
                ps[:].rearrange("p (h d) -> p h d", h=HL),
            )

        def o_group_gen(tb, half):
            """Output projection for one [128 tok, 512 out] block."""
            ps = psW.tile([128, 512], F32, tag="w", name="f_ps")
            for fc in range(4):
                nc.tensor.matmul(
                    ps[:],
                    otn_sb[fc][:, tb * 128:(tb + 1) * 128],
                    wo4[:, fc, half * 512:(half + 1) * 512],
                    start=(fc == 0), stop=(fc == 3),
                )
                if fc == 1:
                    yield
            fin = fin_p.tile([128, 512], BF16, tag="fin", name="fin")
            nc.vector.tensor_copy(fin[:], ps[:])
            nc.sync.dma_start(
                y_t[tb * 128:(tb + 1) * 128, half * 512:(half + 1) * 512], fin[:]
            )

        def run_gen(g):
            for _ in g:
                pass

        def kq_tile(w, dst, fc, tg):
            run_gen(kq_tile_gen(w, dst, fc, tg))

        def v_tile(c):
            run_gen(v_tile_gen(c))

        def o_group(tb, half):
            run_gen(o_group_gen(tb, half))

        # ---- filler schedule: emit each tile as LATE as its deadline
        # allows, so PE filler lands in the Act-bound late query groups.
        # Fillers carry a due-slot s = qg*8 + h (due after head h of group
        # qg); the attention pair loop pops fillers whose due-slot has
        # been reached, weaving them between exp and AV so they cover the
        # scalar-engine latency.
        slots = [[[] for _ in range(HL)] for _ in range(4)]

        def KQ(fc, tg):
            return [lambda: kq_tile_gen("wk", kt_sb, fc, tg),
                    lambda: kq_tile_gen("wq", qt_sb, fc, tg)]

        def V(c):
            return [lambda c=c: v_tile_gen(c)]

        def Og(tb, half):
            return [lambda: o_group_gen(tb, half)]

        # kq(fc, tg) is used at (qg=tg, h=2fc); emit two h-slots earlier.
        slots[0][0] += KQ(1, 0)
        slots[0][1] += KQ(2, 0)
        slots[0][3] += KQ(3, 0)
        for tg in (1, 2, 3):
            slots[tg - 1][5] += KQ(0, tg)
            slots[tg - 1][7] += KQ(1, tg)
            slots[tg][1] += KQ(2, tg)
            slots[tg][3] += KQ(3, tg)
        # V(c) is used at (qg=c//4, h0); emit during the previous group.
        for tg in (1, 2, 3):
            for i in range(4):
                slots[tg - 1][3 + i] += V(4 * tg + i)
        # output projection of qg woven through qg+1 (h1..h7)
        for qg in range(3):
            for tb in range(4):
                slots[qg + 1][1 + tb] += Og(4 * qg + tb, 0)
                slots[qg + 1][min(7, 2 + tb)] += Og(4 * qg + tb, 1)

        fill_q = []  # (due_slot, thunk) in emission order
        for qg in range(4):
            for h in range(HL):
                for th in slots[qg][h]:
                    fill_q.append((qg * 8 + h, th))
        fill_q.reverse()  # pop from the end

        # ---- pre-phase: minimum tiles for (qg0, h0..h1) -----------------
        kq_tile("wk", kt_sb, 0, 0)
        kq_tile("wq", qt_sb, 0, 0)
        for c in range(4):
            v_tile(c)

        # ---- attention ---------------------------------------------------
        fill_state = {"gen": None}

        def pump(s_cur, chunks=1):
            """Advance filler emission by ~`chunks` matmul groups."""
            while chunks > 0:
                if fill_state["gen"] is None:
                    if fill_q and fill_q[-1][0] <= s_cur:
                        fill_state["gen"] = fill_q.pop()[1]()
                    else:
                        return
                try:
                    next(fill_state["gen"])
                except StopIteration:
                    fill_state["gen"] = None
                chunks -= 1

        pend_t = {"v": None}  # deferred transposes (give DVE normalize slack)

        def emit_transposes(fc, qg, nt4):
            for qb in range(4):
                pt = psO.tile([128, 128], BF16, tag="o", name="pst")
                nc.tensor.transpose(pt[:], nt4[:, qb, :], ident[:])
                nc.vector.tensor_copy(
                    otn_sb[fc][:, qg * 512 + qb * 128: qg * 512 + (qb + 1) * 128],
                    pt[:],
                )

        def flush_transposes():
            if pend_t["v"] is not None:
                emit_transposes(*pend_t["v"])
                pend_t["v"] = None

        def attention(hl, qg, carry):
            """Emit one head's attention; returns a closure with the tail
            work (last AVs + normalize) that the NEXT head runs after its
            first S/exp, keeping the scalar engine fed across head
            boundaries."""
            fc, hr = hl // 2, (hl % 2) * 64
            s_cur = qg * 8 + hl
            nch = 4 * qg + 4
            # o_ps is allocated lazily at the first AV so the deferred
            # transposes (same psO pool) never race its buffer slot
            o4c = [None]

            def get_o4():
                if o4c[0] is None:
                    o_ps = psO.tile([128, 260], F32, tag="o", name="o_ps")
                    o4c[0] = o_ps[:].rearrange("p (q x) -> p q x", q=4)
                return o4c[0]

            prev = None  # (p_tile, base_c)

            def do_av(p_tile, base_c):
                # all four 65-col regions share one PSUM bank: exactly one
                # start (zeroes the bank) and one stop for the whole tile
                o4 = get_o4()
                for cc in (base_c, base_c + 1):
                    hh = cc - base_c
                    M = max(0, cc - 4 * qg)
                    for qb in range(M, 4):
                        nc.tensor.matmul(
                            o4[:, qb, :],
                            p_tile[:, hh * 512 + qb * 128: hh * 512 + (qb + 1) * 128],
                            va_sb[cc][:, hl, :],
                            start=(cc == 0 and qb == 0),
                            stop=(cc == nch - 1 and qb == 3),
                        )

            for base_c in range(0, nch, 2):
                s_ps = psS.tile([128, 1024], F32, tag="s", name="s_ps")
                p_tile = p_p.tile([128, 1024], BF16, tag="p", name="p_tile")
                Ms = []
                for hh in (0, 1):
                    cc = base_c + hh
                    M = max(0, cc - 4 * qg)
                    Ms.append(M)
                    nc.tensor.matmul(
                        s_ps[:, hh * 512 + M * 128: (hh + 1) * 512],
                        kt_sb[fc][hr:hr + 64, cc * 128:(cc + 1) * 128],
                        qt_sb[fc][hr:hr + 64, qg * 512 + M * 128:(qg + 1) * 512],
                        start=True, stop=True,
                    )
                if Ms[0] == Ms[1]:
                    M = Ms[0]
                    s_act = s_ps[:].rearrange("p (h x) -> p h x", h=2)[
                        :, :, M * 128: 512]
                    p_act = p_tile[:].rearrange("p (h x) -> p h x", h=2)[
                        :, :, M * 128: 512]
                    nc.scalar.activation(p_act, s_act, Exp, scale=1.0 / (8 * WS * WS))
                else:
                    for hh in (0, 1):
                        M = Ms[hh]
                        sl = slice(hh * 512 + M * 128, (hh + 1) * 512)
                        nc.scalar.activation(
                            p_tile[:, sl], s_ps[:, sl], Exp, scale=1.0 / (8 * WS * WS))
                # diagonal-block mask multiply
                for hh in (0, 1):
                    cc = base_c + hh
                    dqb = cc - 4 * qg
                    if 0 <= dqb < 4:
                        sl = slice(hh * 512 + dqb * 128, hh * 512 + (dqb + 1) * 128)
                        nc.vector.tensor_tensor(
                            p_tile[:, sl], p_tile[:, sl], em_sb[:], mult)
                # run the previous head's tail right after this head's
                # first S/exp is in flight, then flush its transposes
                if base_c == 0 and carry is not None:
                    carry()
                if base_c == 2:
                    flush_transposes()
                # weave filler between exp and the dependent AVs so PE
                # covers the scalar-engine latency without starving it
                pump(s_cur, chunks=1 + (qg >= 2))
                if prev is not None:
                    do_av(*prev)
                prev = (p_tile, base_c)

            last = prev

            def tail():
                do_av(*last)
                # normalize: rec of denominators (col 64 of each 65-group)
                o4 = get_o4()
                rec4 = rec_p.tile([128, 4], F32, tag="rec", name="rec4")
                nc.vector.reciprocal(rec4[:].unsqueeze(-1), o4[:, :, 64:65])
                if hl % 2 == 0:
                    nt4 = nt_p.tile([128, 4, 128], BF16, tag="nt", name="nt4")
                    attention.nt4 = nt4
                else:
                    nt4 = attention.nt4
                nc.vector.tensor_tensor(
                    nt4[:, :, hr:hr + 64],
                    o4[:, :, 0:64],
                    rec4[:].unsqueeze(-1).broadcast_to((128, 4, 64)),
                    mult,
                )
                if hl % 2 == 1:
                    pend_t["v"] = (fc, qg, nt4)

            return tail

        carry = None
        for qg in range(4):
            for hl in range(HL):
                carry = attention(hl, qg, carry)
                # deadline drain with one-slot grace (every filler has >=2
                # slots of slack), keeping the head boundary free for the
                # next head's S/exp
                pump(qg * 8 + hl - 1, chunks=999)
        pump(999, chunks=9999)
        carry()
        flush_transposes()
        for tb in range(4):
            o_group(12 + tb, 0)
            o_group(12 + tb, 1)

        ctx.__exit__(None, None, None)

    nc.compile()
    return nc


# --------------------------------------------------------------------------
# host-side prep for the fast path
# --------------------------------------------------------------------------

def _q8(a):
    hi = a.astype(f8)
    lo = (a - hi.astype(np.float32)).astype(f8)
    return hi, lo


def _dr_layout(a):
    """[1024, N] contraction-major -> [512, 2N] DoubleRow layout.

    Row r = j*128 + p holds contraction indices d = 256j + 128i + p in
    column halves i = 0, 1 (per 128-row block).
    """
    n = a.shape[1]
    return np.ascontiguousarray(
        a.reshape(NJ, 2, 128, n).transpose(0, 2, 1, 3).reshape(512, 2 * n)
    )


def _dr_layout_x(a):
    """[1024, T] -> [128, NJ*2*T] partition-major DoubleRow layout."""
    n = a.shape[1]
    return np.ascontiguousarray(
        a.reshape(NJ, 2, 128, n).transpose(2, 0, 1, 3).reshape(128, NJ * 2 * n)
    )


def _prep_fast_inputs(x, wq, wk, wv, wo):
    ins = []
    # p_tile is [key, query]: allowed iff key <= query -> upper triangular
    em = np.triu(np.ones((128, 128), dtype=np.float32)).astype(bf)
    xq = []
    for b in range(B):
        xh, xl = _q8(x[b].T)  # [1024, 2048]
        xq.append((_dr_layout_x(xh), _dr_layout_x(xl)))
    wqs = []
    for g in range(2):
        d = {}
        for name, w in (("wq", wq), ("wk", wk), ("wv", wv)):
            wt = np.ascontiguousarray(w[g * FG:(g + 1) * FG, :].T) * WS  # [1024, 512]
            hi, lo = _q8(wt)
            d[name + "h"] = _dr_layout(hi)
            d[name + "l"] = _dr_layout(lo)
        d["wo"] = np.ascontiguousarray(wo[:, g * FG:(g + 1) * FG].T / WS).astype(bf)
        wqs.append(d)
    for i in range(8):
        b, g = i // 2, i % 2
        im = {"xh": xq[b][0], "xl": xq[b][1], "em": em}
        im.update(wqs[g])
        ins.append(im)
    return ins


# --------------------------------------------------------------------------
# legacy general/nomask path (baseline kernel, kept as fallback)
# --------------------------------------------------------------------------

TQ = 1024
NDC = DIM // 128


def _causal_sched():
    sched = []
    for a in (0, 1):
        for c in range(8 * a + 8):
            sched.append((a, c, (c - 8 * a) // 2))
    return sched


CAUSAL_SCHED = _causal_sched()


def _build_legacy(variant, reps=1):
    """variant: 'general' | 'nomask' (original baseline kernel)."""
    nc = bacc.Bacc("TRN2", target_bir_lowering=False, debug=False, num_devices=8)

    xT = nc.dram_tensor("xT", [DIM, T], BF16, kind="ExternalInput").ap()
    xqT = nc.dram_tensor("xqT", [DIM, TQ], BF16, kind="ExternalInput").ap()
    wqT = nc.dram_tensor("wqT", [DIM, DIM], BF16, kind="ExternalInput").ap()
    wkT = nc.dram_tensor("wkT", [DIM, DIM], BF16, kind="ExternalInput").ap()
    wvT = nc.dram_tensor("wvT", [DIM, DIM], BF16, kind="ExternalInput").ap()
    woT = nc.dram_tensor("woT", [DIM, DIM], BF16, kind="ExternalInput").ap()
    if variant == "general":
        em = nc.dram_tensor("em", [T, TQ], BF16, kind="ExternalInput").ap()
    else:
        em = None
    out = nc.dram_tensor("out", [TQ, DIM], F32, kind="ExternalOutput").ap()

    with tile.TileContext(nc) as tc:
      for _rep in range(reps):
        ctx = ExitStack()
        ctx.__enter__()
        Exp = mybir.ActivationFunctionType.Exp
        mult = mybir.AluOpType.mult

        qt_p = ctx.enter_context(tc.tile_pool(name="qt", bufs=1))
        kt_p = ctx.enter_context(tc.tile_pool(name="kt", bufs=1))
        va_p = ctx.enter_context(tc.tile_pool(name="va", bufs=1))
        misc_p = ctx.enter_context(tc.tile_pool(name="misc", bufs=1))
        psS = ctx.enter_context(tc.tile_pool(name="psS", bufs=2, space="PSUM"))
        psO = ctx.enter_context(tc.tile_pool(name="psO", bufs=1, space="PSUM"))
        psB = ctx.enter_context(tc.tile_pool(name="psB", bufs=1, space="PSUM"))
        psF = ctx.enter_context(tc.tile_pool(name="psF", bufs=1, space="PSUM"))

        qt_sb = [qt_p.tile([128, TQ], BF16, tag=f"qt{i}", name=f"qt{i}") for i in range(NDC)]
        kt_sb = [kt_p.tile([128, T], BF16, tag=f"kt{i}", name=f"kt{i}") for i in range(NDC)]
        va_sb = [va_p.tile([128, H * 65], BF16, tag=f"va{i}", name=f"va{i}") for i in range(NTC)]
        ones_sb = misc_p.tile([128, 64], F32, tag="ones", name="ones")
        nc.vector.memset(ones_sb[64:65, :], 1.0)

        with tc.tile_pool(name="xin", bufs=1) as x_p, tc.tile_pool(
            name="win", bufs=1
        ) as w_p:
            xt_sb = [x_p.tile([128, T], BF16, tag=f"xt{i}", name=f"xt{i}") for i in range(NDC)]
            xq_sb = [x_p.tile([128, TQ], BF16, tag=f"xq{i}", name=f"xq{i}") for i in range(NDC)]
            wq_sb = [w_p.tile([128, DIM], BF16, tag=f"wq{i}", name=f"wq{i}") for i in range(NDC)]
            wk_sb = [w_p.tile([128, DIM], BF16, tag=f"wk{i}", name=f"wk{i}") for i in range(NDC)]
            wv_sb = [w_p.tile([128, DIM], BF16, tag=f"wv{i}", name=f"wv{i}") for i in range(NDC)]
            for i in range(NDC):
                s = slice(i * 128, (i + 1) * 128)
                nc.sync.dma_start(wv_sb[i][:], wvT[s, :])
                nc.sync.dma_start(xt_sb[i][:], xT[s, :])
            for i in range(NDC):
                s = slice(i * 128, (i + 1) * 128)
                nc.sync.dma_start(wq_sb[i][:], wqT[s, :])
                nc.sync.dma_start(xq_sb[i][:], xqT[s, :])
            for i in range(NDC):
                s = slice(i * 128, (i + 1) * 128)
                nc.sync.dma_start(wk_sb[i][:], wkT[s, :])

            for c in range(NTC):
                v4 = va_sb[c][:].rearrange("p (q t x) -> p q t x", q=8, t=2)
                nc.vector.memset(v4[:, :, 0:2, 64:65], 1.0)

            for c in range(NTC):
                csl = slice(c * 128, (c + 1) * 128)
                ps = psS.tile([128, 1024], F32, tag="s", name="v_ps")
                for n in range(2):
                    nsl = slice(n * 512, (n + 1) * 512)
                    for dc in range(NDC):
                        nc.tensor.matmul(
                            ps[:, nsl],
                            xt_sb[dc][:, csl],
                            wv_sb[dc][:, nsl],
                            start=(dc == 0),
                            stop=(dc == NDC - 1),
                        )
                v4 = va_sb[c][:].rearrange("p (q t x) -> p q t x", q=8, t=2)
                s4 = ps[:].rearrange("p (q t x) -> p q t x", q=8, t=2)
                nc.vector.tensor_copy(v4[:, :, 0:2, 0:64], s4[:, :, 0:2, :])

            for oc in range(NDC):
                osl = slice(oc * 128, (oc + 1) * 128)
                ps = psS.tile([128, 1024], F32, tag="s", name="q_ps")
                for n in range(2):
                    nsl = slice(n * 512, (n + 1) * 512)
                    for dc in range(NDC):
                        nc.tensor.matmul(
                            ps[:, nsl],
                            wq_sb[dc][:, osl],
                            xq_sb[dc][:, nsl],
                            start=(dc == 0),
                            stop=(dc == NDC - 1),
                        )
                nc.scalar.copy(qt_sb[oc][:], ps[:])
                for m in range(2):
                    ps = psS.tile([128, 1024], F32, tag="s", name="k_ps")
                    for n in range(2):
                        nsl = slice((2 * m + n) * 512, (2 * m + n + 1) * 512)
                        psl = slice(n * 512, (n + 1) * 512)
                        for dc in range(NDC):
                            nc.tensor.matmul(
                                ps[:, psl],
                                wk_sb[dc][:, osl],
                                xt_sb[dc][:, nsl],
                                start=(dc == 0),
                                stop=(dc == NDC - 1),
                            )
                    nc.scalar.copy(
                        kt_sb[oc][:, m * 1024: (m + 1) * 1024], ps[:]
                    )

        em_p = ctx.enter_context(tc.tile_pool(name="em", bufs=1))
        wo_p = ctx.enter_context(tc.tile_pool(name="wo", bufs=1))
        otn_p = ctx.enter_context(tc.tile_pool(name="otn", bufs=1))
        p_p = ctx.enter_context(tc.tile_pool(name="pp", bufs=4))
        osb_p = ctx.enter_context(tc.tile_pool(name="osb", bufs=2))
        nrm_p = ctx.enter_context(tc.tile_pool(name="nrm", bufs=2))
        fin_p = ctx.enter_context(tc.tile_pool(name="fin", bufs=2))

        if variant == "general":
            em_sb = [em_p.tile([128, TQ], BF16, tag=f"em{i}", name=f"emt{i}") for i in range(NTC)]
            for c in range(NTC):
                nc.sync.dma_start(em_sb[c][:], em[c * 128: (c + 1) * 128, :])
        wo_sb = [wo_p.tile([128, DIM], BF16, tag=f"wo{i}", name=f"wot{i}") for i in range(NDC)]
        for i in range(NDC):
            nc.sync.dma_start(wo_sb[i][:], woT[i * 128: (i + 1) * 128, :])
        otn_sb = [otn_p.tile([128, TQ], BF16, tag=f"otn{i}", name=f"otn{i}") for i in range(NDC)]
        otn1_sb = [otn_p.tile([64, TQ], BF16, tag=f"otn1{i}", name=f"otn1{i}") for i in range(NDC)]

        for a in (0, 1):
            nA = NTC
            asl = slice(a * 512, (a + 1) * 512)
            for hp in range(NDC):
                o_pair = psO.tile([128, 1024], F32, tag="opair", name="o_pair")
                for c in range(nA):
                    csl = slice(c * 128, (c + 1) * 128)
                    s_ps = psS.tile([128, 1024], F32, tag="s", name="s_ps")
                    nc.tensor.matmul(
                        s_ps[:, 0:512],
                        kt_sb[hp][0:64, csl],
                        qt_sb[hp][0:64, asl],
                        start=True,
                        stop=True,
                    )
                    nc.tensor.matmul(
                        s_ps[:, 512:1024],
                        kt_sb[hp][64:128, csl],
                        qt_sb[hp][64:128, asl],
                        start=True,
                        stop=True,
                        tile_position=(64, 0),
                    )
                    p_pair = p_p.tile([128, 1024], BF16, tag="p", name="p_pair")
                    nc.scalar.activation(p_pair[:], s_ps[:], Exp)
                    if variant == "general":
                        for half in (0, 1):
                            psl = slice(half * 512, half * 512 + 512)
                            nc.vector.tensor_mul(
                                p_pair[:, psl], p_pair[:, psl], em_sb[c][:, asl]
                            )
                    va4 = va_sb[c][:].rearrange("p (q t x) -> p q t x", q=8, t=2)
                    nc.tensor.matmul(
                        o_pair[0:65, 0:512],
                        va4[:, hp, 0, :],
                        p_pair[:, 0:512],
                        start=(c == 0),
                        stop=(c == nA - 1),
                    )
                    nc.tensor.matmul(
                        o_pair[0:65, 512:1024],
                        va4[:, hp, 1, :],
                        p_pair[:, 512:1024],
                        start=(c == 0),
                        stop=(c == nA - 1),
                    )
                o_sb = osb_p.tile([128, 1024], F32, tag="osb", name="o_sb")
                nc.vector.tensor_copy(o_sb[:], o_pair[:])
                rec = nrm_p.tile([128, 1024], mybir.dt.float32r, tag="rec", name="rec")
                with nc.allow_low_precision(reason="f32r recip for denom broadcast"):
                    nc.vector.reciprocal(rec[64:65, 0:1024], o_sb[64:65, 0:1024])
                b0 = psB.tile([64, 512], F32, tag="b", name="b0")
                b1 = psB.tile([64, 512], F32, tag="b", name="b1")
                nc.tensor.matmul(
                    b0[:],
                    ones_sb[64:65, :].bitcast(mybir.dt.float32r),
                    rec[64:65, 0:512],
                    start=True,
                    stop=True,
                    tile_position=(64, 0),
                )
                nc.tensor.matmul(
                    b1[:],
                    ones_sb[64:65, :].bitcast(mybir.dt.float32r),
                    rec[64:65, 512:1024],
                    start=True,
                    stop=True,
                    tile_position=(64, 0),
                )
                nc.vector.tensor_tensor(
                    otn_sb[hp][0:64, asl], o_sb[0:64, 0:512], b0[:], mult
                )
                nc.vector.tensor_tensor(
                    otn1_sb[hp][0:64, asl], o_sb[0:64, 512:1024], b1[:], mult
                )
                nc.sync.dma_start(otn_sb[hp][64:128, asl], otn1_sb[hp][0:64, asl])

            for tt in range(4 * a, 4 * a + 4):
                tsl = slice((tt % 4) * 128 + a * 512, (tt % 4) * 128 + a * 512 + 128)
                fin = fin_p.tile([128, 1024], F32, tag="fin", name="fin")
                for n in range(2):
                    nsl = slice(n * 512, (n + 1) * 512)
                    f_ps = psF.tile([128, 512], F32, tag="fwo", name="f_ps")
                    for hp in range(NDC):
                        nc.tensor.matmul(
                            f_ps[:],
                            otn_sb[hp][:, tsl],
                            wo_sb[hp][:, nsl],
                            start=(hp == 0),
                            stop=(hp == NDC - 1),
                        )
                    nc.vector.tensor_copy(fin[:, nsl], f_ps[:])
                nc.sync.dma_start(out[tt * 128: (tt + 1) * 128, :], fin[:])

        ctx.__exit__(None, None, None)

    nc.compile()
    return nc


_NC_CACHE = {}


def _get_nc(variant):
    if variant not in _NC_CACHE:
        if variant == "causal":
            _NC_CACHE[variant] = _build_fast()
        else:
            _NC_CACHE[variant] = _build_legacy(variant)
    return _NC_CACHE[variant]


def _kernel_legacy(x, emask, wq, wk, wv, wo, variant):
    scale = 1.0 / np.sqrt(HD)
    wqT = np.ascontiguousarray((wq * scale).T).astype(bf)
    wkT = np.ascontiguousarray(wk.T).astype(bf)
    wvT = np.ascontiguousarray(wv.T).astype(bf)
    woT = np.ascontiguousarray(wo.T).astype(bf)

    perms = {}
    for p in (0, 1):
        perms[p] = np.concatenate(
            [np.arange(128) + 128 * j for j in range(p, 16, 2)]
        )

    in_maps = []
    for i in range(8):
        b, p = i // 2, i % 2
        perm = perms[p]
        xb = x[b]
        im = {
            "xT": np.ascontiguousarray(xb.T).astype(bf),
            "xqT": np.ascontiguousarray(xb[perm].T).astype(bf),
            "wqT": wqT,
            "wkT": wkT,
            "wvT": wvT,
            "woT": woT,
        }
        if variant == "general":
            im["em"] = np.ascontiguousarray(emask.T[:, perm]).astype(bf)
        in_maps.append(im)

    nc = _get_nc(variant)
    res = run_bass_kernel_spmd(nc, in_maps, core_ids=list(range(8)))

    out_full = np.empty((B, T, DIM), dtype=np.float32)
    for i in range(8):
        b, p = i // 2, i % 2
        out_full[b, perms[p]] = res.results[i]["out"]
    return out_full


def kernel(x, mask, wq, wk, wv, wo):
    x = np.asarray(x, dtype=np.float32)
    mask = np.asarray(mask, dtype=np.float32)
    wq = np.asarray(wq, dtype=np.float32)
    wk = np.asarray(wk, dtype=np.float32)
    wv = np.asarray(wv, dtype=np.float32)
    wo = np.asarray(wo, dtype=np.float32)

    m2 = mask[0, 0]
    emask = np.exp(np.minimum(m2, 60.0)).astype(np.float32)
    tril = np.tril(np.ones((T, T), dtype=np.float32))
    if np.array_equal(emask, tril):
        variant = "causal"
    elif np.all(m2 == 0.0):
        variant = "nomask"
    else:
        variant = "general"

    if variant != "causal":
        return _kernel_legacy(x, emask, wq, wk, wv, wo, variant)

    in_maps = _prep_fast_inputs(x, wq, wk, wv, wo)
    nc = _get_nc("causal")
    res = run_bass_kernel_spmd(nc, in_maps, core_ids=list(range(8)))

    out_full = np.empty((B, T, DIM), dtype=np.float32)
    for b in range(B):
        y0 = res.results[2 * b]["y"].astype(np.float32)
        y1 = res.results[2 * b + 1]["y"].astype(np.float32)
        out_full[b] = y0 + y1
    return out_full


# revision 38
# speedup vs baseline: 1.2525x; 1.0058x over previous
"""Multi-head attention kernel for 8 Trainium2 NeuronCores.

Problem: B=4, T=2048, DIM=1024, 16 heads, head_dim=64, additive causal mask.
  q,k,v = x@W{q,k,v}.T ; attn = softmax(q k^T/8 + mask) ; out = (attn v)@Wo.T

Sharding (no collectives): core i handles batch i//2 and head-group i%2
(8 heads).  Each core projects q/k/v for its 8 heads only (512 features,
no duplicated projection work), runs full causal attention for those heads,
and computes a partial output projection (contraction over its 512
features).  The host sums the two partial outputs per batch while
unsharding.

On-chip math:
 - Projections run as fp8(e4m3) DoubleRow matmuls (K=256 per instr, 0.5
   cyc/row) with a hi+lo 3-term split (x_hi*w_hi + x_lo*w_hi + x_hi*w_lo)
   for near-bf16 accuracy at 2x bf16 speed.  Weights are pre-scaled by 32
   so hi values sit in e4m3's sweet spot; the scale is folded into the
   exp() scale and into Wo.
 - Scores S[k,q] accumulate in PSUM fp32; exp via scalar engine with
   scale=1/(8*32*32); causal handled by skipping fully-masked 128-blocks
   and one tril-mask multiply per diagonal block.
 - AV uses P as the stationary operand and [V|1] as the moving operand
   (65 cols), giving full PE utilization and the softmax denominator for
   free in column 64.  Attention output lands as [q,hd] per (head,
   q-block), is normalized on DVE (denominator reciprocal broadcast), and
   PE-transposed back to [hd, q] for the output projection.
"""

import sys
import numpy as np

sys.path.insert(0, "/opt/trn_rl_repo")

import ml_dtypes  # noqa: E402
from contextlib import ExitStack  # noqa: E402
from concourse import bass, bacc, tile  # noqa: E402
from concourse.bass_utils import run_bass_kernel_spmd  # noqa: E402
from concourse.masks import make_identity  # noqa: E402

mybir = bass.mybir

B, T, DIM, H, HD = 4, 2048, 1024, 16, 64
HL = 8             # heads per core (head-group)
FG = 512           # features per core (HL * HD)
NJ = 4             # DoubleRow contraction chunks (256 each)
NTC = T // 128     # 16 key chunks
BF16 = mybir.dt.bfloat16
F8 = mybir.dt.float8e4
F32 = mybir.dt.float32
WS = 32.0          # weight pre-scale for e4m3
f8 = ml_dtypes.float8_e4m3fn
bf = ml_dtypes.bfloat16


# --------------------------------------------------------------------------
# fast causal path
# --------------------------------------------------------------------------

def _build_fast(reps=1):
    nc = bacc.Bacc("TRN2", target_bir_lowering=False, debug=False, num_devices=8)
    DR = mybir.MatmulPerfMode.DoubleRow
    Exp = mybir.ActivationFunctionType.Exp
    mult = mybir.AluOpType.mult

    xh_t = nc.dram_tensor("xh", [128, NJ * 2 * T], F8, kind="ExternalInput").ap()
    xl_t = nc.dram_tensor("xl", [128, NJ * 2 * T], F8, kind="ExternalInput").ap()
    w_t = {}
    for w in ("wq", "wk", "wv"):
        for p in ("h", "l"):
            w_t[w + p] = nc.dram_tensor(w + p, [512, 1024], F8, kind="ExternalInput").ap()
    wo_t = nc.dram_tensor("wo", [FG, DIM], BF16, kind="ExternalInput").ap()
    em_t = nc.dram_tensor("em", [128, 128], BF16, kind="ExternalInput").ap()
    y_t = nc.dram_tensor("y", [T, DIM], BF16, kind="ExternalOutput").ap()

    with tile.TileContext(nc) as tc:
      for _rep in range(reps):
        ctx = ExitStack()
        ctx.__enter__()

        # ---- pools -------------------------------------------------------
        x_p = ctx.enter_context(tc.tile_pool(name="xp", bufs=1))
        w_p = ctx.enter_context(tc.tile_pool(name="wp", bufs=1))
        wo_p = ctx.enter_context(tc.tile_pool(name="wop", bufs=1))
        kt_p = ctx.enter_context(tc.tile_pool(name="ktp", bufs=1))
        qt_p = ctx.enter_context(tc.tile_pool(name="qtp", bufs=1))
        va_p = ctx.enter_context(tc.tile_pool(name="vap", bufs=1))
        misc_p = ctx.enter_context(tc.tile_pool(name="miscp", bufs=1))
        otn_p = ctx.enter_context(tc.tile_pool(name="otnp", bufs=1))
        p_p = ctx.enter_context(tc.tile_pool(name="pp", bufs=6))
        nt_p = ctx.enter_context(tc.tile_pool(name="ntp", bufs=2))
        rec_p = ctx.enter_context(tc.tile_pool(name="recp", bufs=2))
        fin_p = ctx.enter_context(tc.tile_pool(name="finp", bufs=3))
        # PSUM: s 2x[128,1024] = 4 banks, o 2x[128,260] = 2 (shared with the
        # [128,128] transpose outputs), w 2x[128,512] = 2  -> 8 banks.
        psS = ctx.enter_context(tc.tile_pool(name="psS", bufs=2, space="PSUM"))
        psO = ctx.enter_context(tc.tile_pool(name="psO", bufs=2, space="PSUM"))
        psW = ctx.enter_context(tc.tile_pool(name="psW", bufs=2, space="PSUM"))

        # ---- persistent SBUF tiles --------------------------------------
        xh4 = x_p.tile([128, NJ, 2, T], F8, tag="xh", name="xh4")
        xl4 = x_p.tile([128, NJ, 2, T], F8, tag="xl", name="xl4")
        w_sb = {}
        for w in ("wq", "wk", "wv"):
            for p in ("h", "l"):
                w_sb[w + p] = w_p.tile(
                    [128, NJ, 2, FG], F8, tag=f"{w}{p}", name=f"{w}{p}"
                )
        wo4 = wo_p.tile([128, 4, DIM], BF16, tag="wo", name="wo4")
        kt_sb = [kt_p.tile([128, T], BF16, tag=f"kt{i}", name=f"kt{i}") for i in range(4)]
        qt_sb = [qt_p.tile([128, T], BF16, tag=f"qt{i}", name=f"qt{i}") for i in range(4)]
        va_sb = [va_p.tile([128, HL, 65], BF16, tag=f"va{i}", name=f"va{i}") for i in range(NTC)]
        otn_sb = [otn_p.tile([128, T], BF16, tag=f"otn{i}", name=f"otn{i}") for i in range(4)]
        em_sb = misc_p.tile([128, 128], BF16, tag="em", name="em_sb")
        ident = misc_p.tile([128, 128], BF16, tag="id", name="ident")
        make_identity(nc, ident[:])

        # ---- input DMAs --------------------------------------------------
        # HWDGE serializes DMA issue (~650ns each), so FEW large DMAs in
        # strict first-needed order: K weights + x token-group 0 first.
        def dma_w(key):
            nc.sync.dma_start(
                w_sb[key][:].rearrange("p a b c -> p a (b c)"),
                w_t[key].rearrange("(j p) f -> p j f", j=NJ),
            )

        def dma_x(p, lo, hi):
            t, x4 = (xh_t, xh4) if p == "h" else (xl_t, xl4)
            src = t[:].rearrange("p (j i t) -> p j i t", j=NJ, i=2)
            nc.sync.dma_start(x4[:, :, :, lo:hi], src[:, :, :, lo:hi])

        dma_w("wkh")
        dma_x("h", 0, 512)
        dma_x("l", 0, 512)
        dma_w("wkl")
        dma_w("wqh")
        dma_w("wql")
        dma_w("wvh")
        dma_w("wvl")
        nc.sync.dma_start(em_sb[:], em_t[:])
        dma_x("h", 512, 1024)
        dma_x("l", 512, 1024)
        dma_x("h", 1024, 2048)
        dma_x("l", 1024, 2048)
        nc.sync.dma_start(
            wo4[:], wo_t[:].rearrange("(a p) o -> p a o", a=4),
        )

        for c in range(NTC):
            nc.vector.memset(va_sb[c][:, :, 64:65], 1.0)

        # ---- projection tile emitters (generators yielding every ~3
        # matmuls so filler can be woven at ~400ns granularity) -----------
        def kq_tile_gen(w, dst, fc, tg):
            """K or Q projection tile: out [128 feat, 512 tok]."""
            ps = psW.tile([128, 512], F32, tag="w", name=f"{w}_ps")
            i = 0
            for (xs, wk) in ((xh4, w + "h"), (xl4, w + "h"),
                             (xh4, w + "l")):
                for j in range(NJ):
                    nc.tensor.matmul(
                        ps[:],
                        w_sb[wk][:, j, :, fc * 128:(fc + 1) * 128],
                        xs[:, j, :, tg * 512:(tg + 1) * 512],
                        start=(i == 0), stop=(i == 3 * NJ - 1), perf_mode=DR,
                    )
                    i += 1
                    if i < 3 * NJ:
                        yield
            nc.vector.tensor_copy(dst[fc][:, tg * 512:(tg + 1) * 512], ps[:])

        def v_tile_gen(c):
            """V projection for key chunk c: out [128 tok, 512 feat]."""
            ps = psW.tile([128, 512], F32, tag="w", name="v_ps")
            i = 0
            for (xs, wk) in ((xh4, "wvh"), (xl4, "wvh"), (xh4, "wvl")):
                for j in range(NJ):
                    nc.tensor.matmul(
                        ps[:],
                        xs[:, j, :, c * 128:(c + 1) * 128],
                        w_sb[wk][:, j],
                        start=(i == 0), stop=(i == 3 * NJ - 1), perf_mode=DR,
                    )
                    i += 1
                    if i < 3 * NJ:
                        yield
            nc.vector.tensor_copy(
                va_sb[c][:, :, 0:64],
                ps[:].rearrange("p (h d) -> p h d", h=HL),
            )

        def o_group_gen(tb, half):
            """Output projection for one [128 tok, 512 out] block."""
            ps = psW.tile([128, 512], F32, tag="w", name="f_ps")
            for fc in range(4):
                nc.tensor.matmul(
                    ps[:],
                    otn_sb[fc][:, tb * 128:(tb + 1) * 128],
                    wo4[:, fc, half * 512:(half + 1) * 512],
                    start=(fc == 0), stop=(fc == 3),
                )
                if fc < 3:
                    yield
            fin = fin_p.tile([128, 512], BF16, tag="fin", name="fin")
            if (tb + half) % 2:
                nc.scalar.copy(fin[:], ps[:])
            else:
                nc.vector.tensor_copy(fin[:], ps[:])
            nc.sync.dma_start(
                y_t[tb * 128:(tb + 1) * 128, half * 512:(half + 1) * 512], fin[:]
            )

        def run_gen(g):
            for _ in g:
                pass

        def kq_tile(w, dst, fc, tg):
            run_gen(kq_tile_gen(w, dst, fc, tg))

        def v_tile(c):
            run_gen(v_tile_gen(c))

        def o_group(tb, half):
            run_gen(o_group_gen(tb, half))

        # ---- filler schedule: emit each tile as LATE as its deadline
        # allows, so PE filler lands in the Act-bound late query groups.
        # Fillers carry a due-slot s = qg*8 + h (due after head h of group
        # qg); the attention pair loop pops fillers whose due-slot has
        # been reached, weaving them between exp and AV so they cover the
        # scalar-engine latency.
        slots = [[[] for _ in range(HL)] for _ in range(4)]

        def KQ(fc, tg):
            return [lambda: kq_tile_gen("wk", kt_sb, fc, tg),
                    lambda: kq_tile_gen("wq", qt_sb, fc, tg)]

        def V(c):
            return [lambda c=c: v_tile_gen(c)]

        def Og(tb, half):
            return [lambda: o_group_gen(tb, half)]

        # kq(fc, tg) is used at (qg=tg, h=2fc); emit two h-slots earlier.
        slots[0][0] += KQ(1, 0)
        slots[0][1] += KQ(2, 0)
        slots[0][3] += KQ(3, 0)
        for tg in (1, 2, 3):
            slots[tg - 1][5] += KQ(0, tg)
            slots[tg - 1][7] += KQ(1, tg)
            slots[tg][1] += KQ(2, tg)
            slots[tg][3] += KQ(3, tg)
        # V(c) is used at (qg=c//4, h0); emit during the previous group.
        for tg in (1, 2, 3):
            for i in range(4):
                slots[tg - 1][3 + i] += V(4 * tg + i)
        # output projection of qg woven through qg+1, biased late where
        # the scalar engine is most saturated
        for qg in range(3):
            for tb in range(4):
                slots[qg + 1][3 + tb] += Og(4 * qg + tb, 0)
                slots[qg + 1][min(7, 4 + tb)] += Og(4 * qg + tb, 1)

        fill_q = []  # (due_slot, thunk) in emission order
        for qg in range(4):
            for h in range(HL):
                for th in slots[qg][h]:
                    fill_q.append((qg * 8 + h, th))
        fill_q.reverse()  # pop from the end

        # ---- pre-phase: minimum tiles for (qg0, h0..h1) -----------------
        kq_tile("wk", kt_sb, 0, 0)
        kq_tile("wq", qt_sb, 0, 0)
        for c in range(4):
            v_tile(c)

        # ---- attention ---------------------------------------------------
        fill_state = {"gen": None}

        def pump(s_cur, chunks=1):
            """Advance filler emission by ~`chunks` matmul groups."""
            while chunks > 0:
                if fill_state["gen"] is None:
                    if fill_q and fill_q[-1][0] <= s_cur:
                        fill_state["gen"] = fill_q.pop()[1]()
                    else:
                        return
                try:
                    next(fill_state["gen"])
                except StopIteration:
                    fill_state["gen"] = None
                chunks -= 1

        pend_t = {"v": None}  # deferred transposes (give DVE normalize slack)

        def emit_transposes(fc, qg, nt4):
            for qb in range(4):
                pt = psO.tile([128, 128], BF16, tag="o", name="pst")
                nc.tensor.transpose(pt[:], nt4[:, qb, :], ident[:])
                nc.vector.tensor_copy(
                    otn_sb[fc][:, qg * 512 + qb * 128: qg * 512 + (qb + 1) * 128],
                    pt[:],
                )

        def flush_transposes():
            if pend_t["v"] is not None:
                emit_transposes(*pend_t["v"])
                pend_t["v"] = None

        def attention(hl, qg, carry):
            """Emit one head's attention; returns a closure with the tail
            work (last AVs + normalize) that the NEXT head runs after its
            first S/exp, keeping the scalar engine fed across head
            boundaries."""
            fc, hr = hl // 2, (hl % 2) * 64
            s_cur = qg * 8 + hl
            nch = 4 * qg + 4
            # o_ps is allocated lazily at the first AV so the deferred
            # transposes (same psO pool) never race its buffer slot
            o4c = [None]

            def get_o4():
                if o4c[0] is None:
                    o_ps = psO.tile([128, 260], F32, tag="o", name="o_ps")
                    o4c[0] = o_ps[:].rearrange("p (q x) -> p q x", q=4)
                return o4c[0]

            prev = None  # (p_tile, base_c)

            def do_av(p_tile, base_c):
                # all four 65-col regions share one PSUM bank: exactly one
                # start (zeroes the bank) and one stop for the whole tile
                o4 = get_o4()
                for cc in (base_c, base_c + 1):
                    hh = cc - base_c
                    M = max(0, cc - 4 * qg)
                    for qb in range(M, 4):
                        nc.tensor.matmul(
                            o4[:, qb, :],
                            p_tile[:, hh * 512 + qb * 128: hh * 512 + (qb + 1) * 128],
                            va_sb[cc][:, hl, :],
                            start=(cc == 0 and qb == 0),
                            stop=(cc == nch - 1 and qb == 3),
                        )

            for base_c in range(0, nch, 2):
                s_ps = psS.tile([128, 1024], F32, tag="s", name="s_ps")
                p_tile = p_p.tile([128, 1024], BF16, tag="p", name="p_tile")
                Ms = []
                for hh in (0, 1):
                    cc = base_c + hh
                    M = max(0, cc - 4 * qg)
                    Ms.append(M)
                    nc.tensor.matmul(
                        s_ps[:, hh * 512 + M * 128: (hh + 1) * 512],
                        kt_sb[fc][hr:hr + 64, cc * 128:(cc + 1) * 128],
                        qt_sb[fc][hr:hr + 64, qg * 512 + M * 128:(qg + 1) * 512],
                        start=True, stop=True,
                    )
                if Ms[0] == Ms[1]:
                    M = Ms[0]
                    s_act = s_ps[:].rearrange("p (h x) -> p h x", h=2)[
                        :, :, M * 128: 512]
                    p_act = p_tile[:].rearrange("p (h x) -> p h x", h=2)[
                        :, :, M * 128: 512]
                    nc.scalar.activation(p_act, s_act, Exp, scale=1.0 / (8 * WS * WS))
                else:
                    for hh in (0, 1):
                        M = Ms[hh]
                        sl = slice(hh * 512 + M * 128, (hh + 1) * 512)
                        nc.scalar.activation(
                            p_tile[:, sl], s_ps[:, sl], Exp, scale=1.0 / (8 * WS * WS))
                # diagonal-block mask multiply
                for hh in (0, 1):
                    cc = base_c + hh
                    dqb = cc - 4 * qg
                    if 0 <= dqb < 4:
                        sl = slice(hh * 512 + dqb * 128, hh * 512 + (dqb + 1) * 128)
                        nc.vector.tensor_tensor(
                            p_tile[:, sl], p_tile[:, sl], em_sb[:], mult)
                # run the previous head's tail right after this head's
                # first S/exp is in flight, then flush its transposes
                if base_c == 0 and carry is not None:
                    carry()
                if base_c == 2:
                    flush_transposes()
                # weave filler between exp and the dependent AVs so PE
                # covers the scalar-engine latency without starving it
                pump(s_cur, chunks=3 + (qg == 3))
                if prev is not None:
                    do_av(*prev)
                prev = (p_tile, base_c)

            last = prev

            def tail():
                do_av(*last)
                # normalize: rec of denominators (col 64 of each 65-group)
                o4 = get_o4()
                rec4 = rec_p.tile([128, 4], F32, tag="rec", name="rec4")
                nc.vector.reciprocal(rec4[:].unsqueeze(-1), o4[:, :, 64:65])
                if hl % 2 == 0:
                    nt4 = nt_p.tile([128, 4, 128], BF16, tag="nt", name="nt4")
                    attention.nt4 = nt4
                else:
                    nt4 = attention.nt4
                nc.vector.tensor_tensor(
                    nt4[:, :, hr:hr + 64],
                    o4[:, :, 0:64],
                    rec4[:].unsqueeze(-1).broadcast_to((128, 4, 64)),
                    mult,
                )
                if hl % 2 == 1:
                    pend_t["v"] = (fc, qg, nt4)

            return tail

        carry = None
        for qg in range(4):
            for hl in range(HL):
                carry = attention(hl, qg, carry)
                # deadline drain with one-slot grace (every filler has >=2
                # slots of slack), keeping the head boundary free for the
                # next head's S/exp
                pump(qg * 8 + hl - 1, chunks=999)
        pump(999, chunks=9999)
        carry()
        flush_transposes()
        for tb in range(4):
            o_group(12 + tb, 0)
            o_group(12 + tb, 1)

        ctx.__exit__(None, None, None)

    nc.compile()
    return nc


# --------------------------------------------------------------------------
# host-side prep for the fast path
# --------------------------------------------------------------------------

def _q8(a):
    hi = a.astype(f8)
    lo = (a - hi.astype(np.float32)).astype(f8)
    return hi, lo


def _dr_layout(a):
    """[1024, N] contraction-major -> [512, 2N] DoubleRow layout.

    Row r = j*128 + p holds contraction indices d = 256j + 128i + p in
    column halves i = 0, 1 (per 128-row block).
    """
    n = a.shape[1]
    return np.ascontiguousarray(
        a.reshape(NJ, 2, 128, n).transpose(0, 2, 1, 3).reshape(512, 2 * n)
    )


def _dr_layout_x(a):
    """[1024, T] -> [128, NJ*2*T] partition-major DoubleRow layout."""
    n = a.shape[1]
    return np.ascontiguousarray(
        a.reshape(NJ, 2, 128, n).transpose(2, 0, 1, 3).reshape(128, NJ * 2 * n)
    )


def _prep_fast_inputs(x, wq, wk, wv, wo):
    ins = []
    # p_tile is [key, query]: allowed iff key <= query -> upper triangular
    em = np.triu(np.ones((128, 128), dtype=np.float32)).astype(bf)
    xq = []
    for b in range(B):
        xh, xl = _q8(x[b].T)  # [1024, 2048]
        xq.append((_dr_layout_x(xh), _dr_layout_x(xl)))
    wqs = []
    for g in range(2):
        d = {}
        for name, w in (("wq", wq), ("wk", wk), ("wv", wv)):
            wt = np.ascontiguousarray(w[g * FG:(g + 1) * FG, :].T) * WS  # [1024, 512]
            hi, lo = _q8(wt)
            d[name + "h"] = _dr_layout(hi)
            d[name + "l"] = _dr_layout(lo)
        d["wo"] = np.ascontiguousarray(wo[:, g * FG:(g + 1) * FG].T / WS).astype(bf)
        wqs.append(d)
    for i in range(8):
        b, g = i // 2, i % 2
        im = {"xh": xq[b][0], "xl": xq[b][1], "em": em}
        im.update(wqs[g])
        ins.append(im)
    return ins


# --------------------------------------------------------------------------
# legacy general/nomask path (baseline kernel, kept as fallback)
# --------------------------------------------------------------------------

TQ = 1024
NDC = DIM // 128


def _causal_sched():
    sched = []
    for a in (0, 1):
        for c in range(8 * a + 8):
            sched.append((a, c, (c - 8 * a) // 2))
    return sched


CAUSAL_SCHED = _causal_sched()


def _build_legacy(variant, reps=1):
    """variant: 'general' | 'nomask' (original baseline kernel)."""
    nc = bacc.Bacc("TRN2", target_bir_lowering=False, debug=False, num_devices=8)

    xT = nc.dram_tensor("xT", [DIM, T], BF16, kind="ExternalInput").ap()
    xqT = nc.dram_tensor("xqT", [DIM, TQ], BF16, kind="ExternalInput").ap()
    wqT = nc.dram_tensor("wqT", [DIM, DIM], BF16, kind="ExternalInput").ap()
    wkT = nc.dram_tensor("wkT", [DIM, DIM], BF16, kind="ExternalInput").ap()
    wvT = nc.dram_tensor("wvT", [DIM, DIM], BF16, kind="ExternalInput").ap()
    woT = nc.dram_tensor("woT", [DIM, DIM], BF16, kind="ExternalInput").ap()
    if variant == "general":
        em = nc.dram_tensor("em", [T, TQ], BF16, kind="ExternalInput").ap()
    else:
        em = None
    out = nc.dram_tensor("out", [TQ, DIM], F32, kind="ExternalOutput").ap()

    with tile.TileContext(nc) as tc:
      for _rep in range(reps):
        ctx = ExitStack()
        ctx.__enter__()
        Exp = mybir.ActivationFunctionType.Exp
        mult = mybir.AluOpType.mult

        qt_p = ctx.enter_context(tc.tile_pool(name="qt", bufs=1))
        kt_p = ctx.enter_context(tc.tile_pool(name="kt", bufs=1))
        va_p = ctx.enter_context(tc.tile_pool(name="va", bufs=1))
        misc_p = ctx.enter_context(tc.tile_pool(name="misc", bufs=1))
        psS = ctx.enter_context(tc.tile_pool(name="psS", bufs=2, space="PSUM"))
        psO = ctx.enter_context(tc.tile_pool(name="psO", bufs=1, space="PSUM"))
        psB = ctx.enter_context(tc.tile_pool(name="psB", bufs=1, space="PSUM"))
        psF = ctx.enter_context(tc.tile_pool(name="psF", bufs=1, space="PSUM"))

        qt_sb = [qt_p.tile([128, TQ], BF16, tag=f"qt{i}", name=f"qt{i}") for i in range(NDC)]
        kt_sb = [kt_p.tile([128, T], BF16, tag=f"kt{i}", name=f"kt{i}") for i in range(NDC)]
        va_sb = [va_p.tile([128, H * 65], BF16, tag=f"va{i}", name=f"va{i}") for i in range(NTC)]
        ones_sb = misc_p.tile([128, 64], F32, tag="ones", name="ones")
        nc.vector.memset(ones_sb[64:65, :], 1.0)

        with tc.tile_pool(name="xin", bufs=1) as x_p, tc.tile_pool(
            name="win", bufs=1
        ) as w_p:
            xt_sb = [x_p.tile([128, T], BF16, tag=f"xt{i}", name=f"xt{i}") for i in range(NDC)]
            xq_sb = [x_p.tile([128, TQ], BF16, tag=f"xq{i}", name=f"xq{i}") for i in range(NDC)]
            wq_sb = [w_p.tile([128, DIM], BF16, tag=f"wq{i}", name=f"wq{i}") for i in range(NDC)]
            wk_sb = [w_p.tile([128, DIM], BF16, tag=f"wk{i}", name=f"wk{i}") for i in range(NDC)]
            wv_sb = [w_p.tile([128, DIM], BF16, tag=f"wv{i}", name=f"wv{i}") for i in range(NDC)]
            for i in range(NDC):
                s = slice(i * 128, (i + 1) * 128)
                nc.sync.dma_start(wv_sb[i][:], wvT[s, :])
                nc.sync.dma_start(xt_sb[i][:], xT[s, :])
            for i in range(NDC):
                s = slice(i * 128, (i + 1) * 128)
                nc.sync.dma_start(wq_sb[i][:], wqT[s, :])
                nc.sync.dma_start(xq_sb[i][:], xqT[s, :])
            for i in range(NDC):
                s = slice(i * 128, (i + 1) * 128)
                nc.sync.dma_start(wk_sb[i][:], wkT[s, :])

            for c in range(NTC):
                v4 = va_sb[c][:].rearrange("p (q t x) -> p q t x", q=8, t=2)
                nc.vector.memset(v4[:, :, 0:2, 64:65], 1.0)

            for c in range(NTC):
                csl = slice(c * 128, (c + 1) * 128)
                ps = psS.tile([128, 1024], F32, tag="s", name="v_ps")
                for n in range(2):
                    nsl = slice(n * 512, (n + 1) * 512)
                    for dc in range(NDC):
                        nc.tensor.matmul(
                            ps[:, nsl],
                            xt_sb[dc][:, csl],
                            wv_sb[dc][:, nsl],
                            start=(dc == 0),
                            stop=(dc == NDC - 1),
                        )
                v4 = va_sb[c][:].rearrange("p (q t x) -> p q t x", q=8, t=2)
                s4 = ps[:].rearrange("p (q t x) -> p q t x", q=8, t=2)
                nc.vector.tensor_copy(v4[:, :, 0:2, 0:64], s4[:, :, 0:2, :])

            for oc in range(NDC):
                osl = slice(oc * 128, (oc + 1) * 128)
                ps = psS.tile([128, 1024], F32, tag="s", name="q_ps")
                for n in range(2):
                    nsl = slice(n * 512, (n + 1) * 512)
                    for dc in range(NDC):
                        nc.tensor.matmul(
                            ps[:, nsl],
                            wq_sb[dc][:, osl],
                            xq_sb[dc][:, nsl],
                            start=(dc == 0),
                            stop=(dc == NDC - 1),
                        )
                nc.scalar.copy(qt_sb[oc][:], ps[:])
                for m in range(2):
                    ps = psS.tile([128, 1024], F32, tag="s", name="k_ps")
                    for n in range(2):
                        nsl = slice((2 * m + n) * 512, (2 * m + n + 1) * 512)
                        psl = slice(n * 512, (n + 1) * 512)
                        for dc in range(NDC):
                            nc.tensor.matmul(
                                ps[:, psl],
                                wk_sb[dc][:, osl],
                                xt_sb[dc][:, nsl],
                                start=(dc == 0),
                                stop=(dc == NDC - 1),
                            )
                    nc.scalar.copy(
                        kt_sb[oc][:, m * 1024: (m + 1) * 1024], ps[:]
                    )

        em_p = ctx.enter_context(tc.tile_pool(name="em", bufs=1))
        wo_p = ctx.enter_context(tc.tile_pool(name="wo", bufs=1))
        otn_p = ctx.enter_context(tc.tile_pool(name="otn", bufs=1))
        p_p = ctx.enter_context(tc.tile_pool(name="pp", bufs=6))
        osb_p = ctx.enter_context(tc.tile_pool(name="osb", bufs=2))
        nrm_p = ctx.enter_context(tc.tile_pool(name="nrm", bufs=2))
        fin_p = ctx.enter_context(tc.tile_pool(name="fin", bufs=2))

        if variant == "general":
            em_sb = [em_p.tile([128, TQ], BF16, tag=f"em{i}", name=f"emt{i}") for i in range(NTC)]
            for c in range(NTC):
                nc.sync.dma_start(em_sb[c][:], em[c * 128: (c + 1) * 128, :])
        wo_sb = [wo_p.tile([128, DIM], BF16, tag=f"wo{i}", name=f"wot{i}") for i in range(NDC)]
        for i in range(NDC):
            nc.sync.dma_start(wo_sb[i][:], woT[i * 128: (i + 1) * 128, :])
        otn_sb = [otn_p.tile([128, TQ], BF16, tag=f"otn{i}", name=f"otn{i}") for i in range(NDC)]
        otn1_sb = [otn_p.tile([64, TQ], BF16, tag=f"otn1{i}", name=f"otn1{i}") for i in range(NDC)]

        for a in (0, 1):
            nA = NTC
            asl = slice(a * 512, (a + 1) * 512)
            for hp in range(NDC):
                o_pair = psO.tile([128, 1024], F32, tag="opair", name="o_pair")
                for c in range(nA):
                    csl = slice(c * 128, (c + 1) * 128)
                    s_ps = psS.tile([128, 1024], F32, tag="s", name="s_ps")
                    nc.tensor.matmul(
                        s_ps[:, 0:512],
                        kt_sb[hp][0:64, csl],
                        qt_sb[hp][0:64, asl],
                        start=True,
                        stop=True,
                    )
                    nc.tensor.matmul(
                        s_ps[:, 512:1024],
                        kt_sb[hp][64:128, csl],
                        qt_sb[hp][64:128, asl],
                        start=True,
                        stop=True,
                        tile_position=(64, 0),
                    )
                    p_pair = p_p.tile([128, 1024], BF16, tag="p", name="p_pair")
                    nc.scalar.activation(p_pair[:], s_ps[:], Exp)
                    if variant == "general":
                        for half in (0, 1):
                            psl = slice(half * 512, half * 512 + 512)
                            nc.vector.tensor_mul(
                                p_pair[:, psl], p_pair[:, psl], em_sb[c][:, asl]
                            )
                    va4 = va_sb[c][:].rearrange("p (q t x) -> p q t x", q=8, t=2)
                    nc.tensor.matmul(
                        o_pair[0:65, 0:512],
                        va4[:, hp, 0, :],
                        p_pair[:, 0:512],
                        start=(c == 0),
                        stop=(c == nA - 1),
                    )
                    nc.tensor.matmul(
                        o_pair[0:65, 512:1024],
                        va4[:, hp, 1, :],
                        p_pair[:, 512:1024],
                        start=(c == 0),
                        stop=(c == nA - 1),
                    )
                o_sb = osb_p.tile([128, 1024], F32, tag="osb", name="o_sb")
                nc.vector.tensor_copy(o_sb[:], o_pair[:])
                rec = nrm_p.tile([128, 1024], mybir.dt.float32r, tag="rec", name="rec")
                with nc.allow_low_precision(reason="f32r recip for denom broadcast"):
                    nc.vector.reciprocal(rec[64:65, 0:1024], o_sb[64:65, 0:1024])
                b0 = psB.tile([64, 512], F32, tag="b", name="b0")
                b1 = psB.tile([64, 512], F32, tag="b", name="b1")
                nc.tensor.matmul(
                    b0[:],
                    ones_sb[64:65, :].bitcast(mybir.dt.float32r),
                    rec[64:65, 0:512],
                    start=True,
                    stop=True,
                    tile_position=(64, 0),
                )
                nc.tensor.matmul(
                    b1[:],
                    ones_sb[64:65, :].bitcast(mybir.dt.float32r),
                    rec[64:65, 512:1024],
                    start=True,
                    stop=True,
                    tile_position=(64, 0),
                )
                nc.vector.tensor_tensor(
                    otn_sb[hp][0:64, asl], o_sb[0:64, 0:512], b0[:], mult
                )
                nc.vector.tensor_tensor(
                    otn1_sb[hp][0:64, asl], o_sb[0:64, 512:1024], b1[:], mult
                )
                nc.sync.dma_start(otn_sb[hp][64:128, asl], otn1_sb[hp][0:64, asl])

            for tt in range(4 * a, 4 * a + 4):
                tsl = slice((tt % 4) * 128 + a * 512, (tt % 4) * 128 + a * 512 + 128)
                fin = fin_p.tile([128, 1024], F32, tag="fin", name="fin")
                for n in range(2):
                    nsl = slice(n * 512, (n + 1) * 512)
                    f_ps = psF.tile([128, 512], F32, tag="fwo", name="f_ps")
                    for hp in range(NDC):
                        nc.tensor.matmul(
                            f_ps[:],
                            otn_sb[hp][:, tsl],
                            wo_sb[hp][:, nsl],
                            start=(hp == 0),
                            stop=(hp == NDC - 1),
                        )
                    nc.vector.tensor_copy(fin[:, nsl], f_ps[:])
                nc.sync.dma_start(out[tt * 128: (tt + 1) * 128, :], fin[:])

        ctx.__exit__(None, None, None)

    nc.compile()
    return nc


_NC_CACHE = {}


def _get_nc(variant):
    if variant not in _NC_CACHE:
        if variant == "causal":
            _NC_CACHE[variant] = _build_fast()
        else:
            _NC_CACHE[variant] = _build_legacy(variant)
    return _NC_CACHE[variant]


def _kernel_legacy(x, emask, wq, wk, wv, wo, variant):
    scale = 1.0 / np.sqrt(HD)
    wqT = np.ascontiguousarray((wq * scale).T).astype(bf)
    wkT = np.ascontiguousarray(wk.T).astype(bf)
    wvT = np.ascontiguousarray(wv.T).astype(bf)
    woT = np.ascontiguousarray(wo.T).astype(bf)

    perms = {}
    for p in (0, 1):
        perms[p] = np.concatenate(
            [np.arange(128) + 128 * j for j in range(p, 16, 2)]
        )

    in_maps = []
    for i in range(8):
        b, p = i // 2, i % 2
        perm = perms[p]
        xb = x[b]
        im = {
            "xT": np.ascontiguousarray(xb.T).astype(bf),
            "xqT": np.ascontiguousarray(xb[perm].T).astype(bf),
            "wqT": wqT,
            "wkT": wkT,
            "wvT": wvT,
            "woT": woT,
        }
        if variant == "general":
            im["em"] = np.ascontiguousarray(emask.T[:, perm]).astype(bf)
        in_maps.append(im)

    nc = _get_nc(variant)
    res = run_bass_kernel_spmd(nc, in_maps, core_ids=list(range(8)))

    out_full = np.empty((B, T, DIM), dtype=np.float32)
    for i in range(8):
        b, p = i // 2, i % 2
        out_full[b, perms[p]] = res.results[i]["out"]
    return out_full


def kernel(x, mask, wq, wk, wv, wo):
    x = np.asarray(x, dtype=np.float32)
    mask = np.asarray(mask, dtype=np.float32)
    wq = np.asarray(wq, dtype=np.float32)
    wk = np.asarray(wk, dtype=np.float32)
    wv = np.asarray(wv, dtype=np.float32)
    wo = np.asarray(wo, dtype=np.float32)

    m2 = mask[0, 0]
    emask = np.exp(np.minimum(m2, 60.0)).astype(np.float32)
    tril = np.tril(np.ones((T, T), dtype=np.float32))
    if np.array_equal(emask, tril):
        variant = "causal"
    elif np.all(m2 == 0.0):
        variant = "nomask"
    else:
        variant = "general"

    if variant != "causal":
        return _kernel_legacy(x, emask, wq, wk, wv, wo, variant)

    in_maps = _prep_fast_inputs(x, wq, wk, wv, wo)
    nc = _get_nc("causal")
    res = run_bass_kernel_spmd(nc, in_maps, core_ids=list(range(8)))

    out_full = np.empty((B, T, DIM), dtype=np.float32)
    for b in range(B):
        y0 = res.results[2 * b]["y"].astype(np.float32)
        y1 = res.results[2 * b + 1]["y"].astype(np.float32)
        out_full[b] = y0 + y1
    return out_full
